# revision 9
# baseline (speedup 1.0000x reference)
"""Trainium2 Bass kernel for nn_MoEEncoderDecoderGPT (moe_routing).

Sharding: 4096 tokens x 512/core over 8 cores (cores 0-3 batch 0, 4-7 batch 1).
One grouped AllGather ([[0-3],[4-7]]) moves ai (token-major), ao^T
(feature-major) and the D-sharded combo weights; router-loss partials are
reduced on host. The per-expert [E,N,H]x[H,D] einsums are collapsed:
  moe_out = (sum_e w_e * LN_e(pre @ A_e^T)) @ (W_oproj @ W_eproj)^T   [combo1]
  shared  = hidden @ W_down^T + 0.1 * adapt @ (W_down @ W_aproj)^T    [combo2]
Big matmuls run in float32r (full PE rate on fp32 data). The BIR verifier
requires fp32r matmul inputs to be *produced* as fp32r, so those tiles are
declared float32r; DVE/ACT reads of them go through a bitcast-to-f32 view.
"""

import contextlib

import numpy as np

import concourse.bass as bass
import concourse.tile as tile_mod
from concourse import bacc
from concourse import mybir
from concourse import bass_utils
from concourse.masks import make_identity
from concourse.vector_clock import ScopedClock

# ---- problem constants --------------------------------------------------
B, S, D = 2, 2048, 1024
E, GS, KTOP = 8, 4, 2
H = 2 * D            # 2048
A = H // 16          # 128
G = E // GS          # 2
N = B * S            # 4096
NC = 8               # cores
NL = N // NC         # 512 tokens per core
DC = D // 128        # 8 d-chunks
HC = H // 128        # 16 h-chunks
TB = NL // 128       # 4 token blocks per core
GRP = 4              # ranks per AllGather group
LN_EPS = 1e-5
R32 = mybir.dt.float32r
F32 = mybir.dt.float32

# cin row layout (width 128 f32): per-rank AllGather contribution
ROWS_AI = NL                      # 512 rows: ai token-major [512,128]
ROWS_AO = NL                      # 512 rows: aoT [128,512] viewed (p x) c
ROWS_C1 = 256                     # combo1T shard [128,256] viewed (p x) c
ROWS_C2 = 256                     # combo2T shard [128,256]
CIN_ROWS = ROWS_AI + ROWS_AO + ROWS_C1 + ROWS_C2   # 1536
OFF_AO = ROWS_AI
OFF_C1 = ROWS_AI + ROWS_AO
OFF_C2 = OFF_C1 + ROWS_C1


# ---- workaround: this walrus build allows only ONE sync-wait per Drain ----
def _patched_drain_and_barrier(self, tick_clock, wait_clock):
    nc = self.nc
    drain_inst = nc.sync.drain()
    wait_clock.add_sem_waits(
        drain_inst.ins, ScopedClock({None: tick_clock.global_clock})
    )
    si = drain_inst.ins.sync_info
    if si is not None and len(si.on_wait) > 1:
        waits = list(si.on_wait)
        ups = list(si.on_update)
        drain_inst.ins.sync_info = mybir.SyncInfo(on_wait=[waits[0]], on_update=[])
        last = drain_inst
        for w in waits[1:]:
            d = nc.sync.drain()
            d.ins.sync_info = mybir.SyncInfo(on_wait=[w], on_update=[])
            last = d
        if ups:
            lsi = last.ins.sync_info
            last.ins.sync_info = mybir.SyncInfo(
                on_wait=list(lsi.on_wait), on_update=ups
            )
    nc.all_engine_barrier()
    assert self.sems is not None
    popped = nc._tile_sem_poison_stack.pop()
    assert popped is self._sem_poison
    nc.clear_and_free_semaphores(list(self.sems.allocated().values()))
    nc.all_engine_barrier()


tile_mod.TileContext._drain_and_barrier = _patched_drain_and_barrier


def _f32(ap):
    """f32 view of a float32r tile for DVE/ACT reads"""
    return ap.bitcast(F32)


def build_program():
    nc = bacc.Bacc("TRN2", target_bir_lowering=False, debug=False,
                   num_devices=NC)
    io = {}
    for name, shape, dt in [
        ("xT", [D, NL], R32), ("wupT", [D, H], R32), ("wgateT", [D, H], R32),
        ("wdownT", [H, D], R32), ("wpreT", [D, A], R32),
        ("wpostT", [H, A], R32), ("waproj", [H, A], R32),
        ("weproj", [H, A], R32), ("wcrhs", [H, 512], R32),
        ("aexpP", [A, E * A], R32), ("wrge", [D, G + GS], R32),
        ("lngb", [A, 2], F32), ("lngeP", [1, E * A], F32),
        ("lnbeP", [1, E * A], F32),
    ]:
        io[name] = nc.dram_tensor(name, shape, dt, kind="ExternalInput").ap()
    io["outT"] = nc.dram_tensor("outT", [D, NL], F32, kind="ExternalOutput").ap()
    io["rpart"] = nc.dram_tensor("rpart", [1, 16], F32,
                                 kind="ExternalOutput").ap()

    with tile_mod.TileContext(nc) as tc:
        _body(nc, tc, io)
    nc.compile()
    return nc


def _body(nc, tc, io):
    AL = mybir.AluOpType
    AF = mybir.ActivationFunctionType

    ctx = contextlib.ExitStack()
    with ctx:
        singles = ctx.enter_context(tc.tile_pool(name="singles", bufs=1))
        big = ctx.enter_context(tc.tile_pool(name="big", bufs=1))
        wpool = ctx.enter_context(tc.tile_pool(name="wpool", bufs=2))
        tmp = ctx.enter_context(tc.tile_pool(name="tmp", bufs=2))
        rtr = ctx.enter_context(tc.tile_pool(name="rtr", bufs=2))
        # PSUM budget: 8 banks = mm(3) + acc(2) + tp(2) + row(1)
        ps_mm = ctx.enter_context(tc.tile_pool(name="ps_mm", bufs=3, space="PSUM"))
        ps_acc = ctx.enter_context(tc.tile_pool(name="ps_acc", bufs=2, space="PSUM"))
        ps_tp = ctx.enter_context(tc.tile_pool(name="ps_tp", bufs=2, space="PSUM"))
        ps_row = ctx.enter_context(tc.tile_pool(name="ps_row", bufs=1, space="PSUM"))
        dram = ctx.enter_context(tc.tile_pool(name="dram", bufs=1, space="DRAM"))
        xph_cm = tc.tile_pool(name="xph", bufs=1)
        xph = xph_cm.__enter__()

        def mm_tile(cols=NL):
            return ps_mm.tile([128, cols], F32, tag="mm", name="mm",
                              padded_shape=[128, NL])

        # ---------- constants / small loads ----------
        ident = singles.tile([128, 128], F32)
        make_identity(nc, ident)
        ones_col = singles.tile([128, 1], R32)
        nc.vector.memset(_f32(ones_col), 1.0)
        ones_row = singles.tile([1, 128], F32)
        nc.vector.memset(ones_row, 1.0)
        eps1 = singles.tile([1, 1], F32)
        nc.vector.memset(eps1, LN_EPS)
        epsP = singles.tile([128, 1], F32)
        nc.vector.memset(epsP, LN_EPS)
        lngb_sb = singles.tile([A, 2], F32)
        nc.gpsimd.dma_start(out=lngb_sb, in_=io["lngb"])
        lnge_bc = singles.tile([128, E * A], F32)
        nc.gpsimd.dma_start(out=lnge_bc, in_=io["lngeP"].to_broadcast([128, E * A]))
        lnbe_bc = singles.tile([128, E * A], F32)
        nc.gpsimd.dma_start(out=lnbe_bc, in_=io["lnbeP"].to_broadcast([128, E * A]))
        wrge_sb = singles.tile([128, DC, G + GS], R32)
        nc.sync.dma_start(out=wrge_sb,
                          in_=io["wrge"].rearrange("(c p) g -> p c g", p=128))
        aexp_sb = singles.tile([128, E * A], R32)
        nc.sync.dma_start(out=aexp_sb, in_=io["aexpP"])
        wpre_sb = singles.tile([128, DC, A], R32)
        nc.sync.dma_start(out=wpre_sb,
                          in_=io["wpreT"].rearrange("(c p) a -> p c a", p=128))

        # xT: [D, NL] -> sbuf [128, DC, NL]
        xT_sb = xph.tile([128, DC, NL], R32)
        nc.sync.dma_start(out=xT_sb,
                          in_=io["xT"].rearrange("(c p) n -> p c n", p=128))

        # DRAM bounce buffers for the collective + sdm row trick
        cin = dram.tile([CIN_ROWS, 128], R32)
        # note: Shared addr space is rejected for 4-rank groups by this stack
        cout = dram.tile([GRP * CIN_ROWS, 128], R32)
        sdm_d = dram.tile([TB, 128], F32)

        # ---------- pre^T = (x @ W_pre.T)^T  (feature-major [A, NL]) ----------
        pre_ps = mm_tile()
        for dc in range(DC):
            nc.tensor.matmul(pre_ps, wpre_sb[:, dc, :], xT_sb[:, dc, :],
                             start=(dc == 0), stop=(dc == DC - 1))
        pre_sb = big.tile([128, NL], R32)
        nc.vector.tensor_copy(out=pre_sb, in_=pre_ps)

        # ---------- partition-major LN (over A=128 partitions) ----------
        def part_ln(src_sb, dst_sb):
            """src_sb/dst_sb are float32r [128, NL]."""
            sums = ps_row.tile([1, NL], F32, tag="row", padded_shape=[1, NL])
            nc.tensor.matmul(sums, ones_col, src_sb, start=True, stop=True)
            sq_sb = tmp.tile([128, NL], R32, tag="lnsq", bufs=1)
            nc.scalar.square(out=sq_sb, in_=_f32(src_sb))
            sqs = ps_row.tile([1, NL], F32, tag="row", padded_shape=[1, NL])
            nc.tensor.matmul(sqs, ones_col, sq_sb, start=True, stop=True)
            pack = tmp.tile([1, 2 * NL], F32, tag="lnpack", bufs=1)
            mu = pack[:, 0:NL]
            rs = pack[:, NL:2 * NL]
            nc.vector.tensor_scalar_mul(out=mu, in0=sums, scalar1=1.0 / 128.0)
            musq = tmp.tile([1, NL], F32, tag="lnmusq", bufs=1)
            nc.scalar.square(out=musq, in_=mu)
            ex2 = tmp.tile([1, NL], F32, tag="lnex2", bufs=1)
            nc.vector.tensor_scalar_mul(out=ex2, in0=sqs, scalar1=1.0 / 128.0)
            var = tmp.tile([1, NL], F32, tag="lnvar", bufs=1)
            nc.vector.tensor_sub(out=var, in0=ex2, in1=musq)
            sd = tmp.tile([1, NL], F32, tag="lnsd", bufs=1)
            nc.scalar.activation(out=sd, in_=var, func=AF.Sqrt,
                                 bias=eps1, scale=1.0)
            nc.vector.reciprocal(out=rs, in_=sd)
            # plain-f32 broadcast matmuls (4 cyc/row, negligible here)
            mu_bc = mm_tile()
            nc.tensor.matmul(mu_bc, ones_row, mu, start=True, stop=True)
            rs_bc = mm_tile()
            nc.tensor.matmul(rs_bc, ones_row, rs, start=True, stop=True)
            t = tmp.tile([128, NL], F32, tag="lnt", bufs=1)
            nc.vector.tensor_sub(out=t, in0=_f32(src_sb), in1=mu_bc)
            nc.vector.tensor_mul(out=t, in0=t, in1=rs_bc)
            nc.vector.tensor_scalar(out=dst_sb, in0=t,
                                    scalar1=lngb_sb[:, 0:1],
                                    scalar2=lngb_sb[:, 1:2],
                                    op0=AL.mult, op1=AL.add)

        # ai^T = LN(pre^T)  (feature-major; scores rhs)
        aiT_sb = big.tile([128, NL], R32)
        part_ln(pre_sb, aiT_sb)

        # ai token-major -> cin rows 0:512
        for tb in range(TB):
            tp = ps_tp.tile([128, 128], F32, tag="tp")
            nc.tensor.transpose(tp, _f32(aiT_sb)[:, tb * 128:(tb + 1) * 128],
                                ident)
            ai_tok = tmp.tile([128, 128], R32, tag="aitok")
            nc.vector.tensor_copy(out=ai_tok, in_=tp)
            nc.sync.dma_start(out=cin[tb * 128:(tb + 1) * 128, :], in_=ai_tok)

        # ---------- router ----------
        lgT_ps = mm_tile()
        for dc in range(DC):
            nc.tensor.matmul(lgT_ps[0:G + GS, :], wrge_sb[:, dc, :],
                             xT_sb[:, dc, :],
                             start=(dc == 0), stop=(dc == DC - 1))
        lgT_sb = xph.tile([G + GS, NL], F32, tag="lgT")
        nc.vector.tensor_copy(out=lgT_sb, in_=lgT_ps[0:G + GS, :])

        dm_all = big.tile([128, TB, E], R32)
        sdm_all = big.tile([128, TB], F32)
        load_ps = ps_row.tile([1, 16], F32, tag="row", padded_shape=[1, NL])
        for tb in range(TB):
            ltp = ps_tp.tile([128, 128], F32, tag="tp")
            nc.tensor.transpose(ltp[:, 0:G + GS],
                                lgT_sb[:, tb * 128:(tb + 1) * 128],
                                ident[0:G + GS, 0:G + GS])
            lt = rtr.tile([128, G + GS], F32, tag="lt")
            nc.vector.tensor_copy(out=lt, in_=ltp[:, 0:G + GS])
            gl0, gl1, el = lt[:, 0:1], lt[:, 1:2], lt[:, 2:6]
            gdiff = rtr.tile([128, 1], F32, tag="gdiff")
            nc.vector.tensor_sub(out=gdiff, in0=gl1, in1=gl0)
            gidx = rtr.tile([128, 1], F32, tag="gidx")
            nc.vector.tensor_scalar(out=gidx, in0=gdiff, scalar1=0.0,
                                    scalar2=None, op0=AL.is_gt)
            gabs = rtr.tile([128, 1], F32, tag="gabs")
            nc.scalar.activation(out=gabs, in_=gdiff, func=AF.Abs)
            gw = rtr.tile([128, 1], F32, tag="gw")
            nc.scalar.activation(out=gw, in_=gabs, func=AF.Sigmoid)
            m4 = rtr.tile([128, 1], F32, tag="m4")
            nc.vector.reduce_max(m4, el, axis=mybir.AxisListType.X)
            negm = rtr.tile([128, 1], F32, tag="negm")
            nc.vector.tensor_scalar_mul(out=negm, in0=m4, scalar1=-1.0)
            eexp = rtr.tile([128, GS], F32, tag="eexp")
            nc.scalar.activation(out=eexp, in_=el, func=AF.Exp,
                                 bias=negm, scale=1.0)
            ssum = rtr.tile([128, 1], F32, tag="ssum")
            nc.vector.reduce_sum(ssum, eexp, axis=mybir.AxisListType.X)
            srec = rtr.tile([128, 1], F32, tag="srec")
            nc.vector.reciprocal(out=srec, in_=ssum)
            ep = rtr.tile([128, GS], F32, tag="ep")
            nc.vector.tensor_scalar_mul(out=ep, in0=eexp, scalar1=srec)
            el8 = rtr.tile([128, 2 * GS], F32, tag="el8")
            nc.vector.tensor_copy(out=el8[:, 0:GS], in_=el)
            nc.vector.tensor_copy(out=el8[:, GS:2 * GS], in_=el)
            rank = rtr.tile([128, GS], F32, tag="rank")
            c2 = rtr.tile([128, GS], F32, tag="c2")
            nc.vector.tensor_tensor(out=rank, in0=el8[:, 1:5], in1=el, op=AL.is_gt)
            nc.vector.tensor_tensor(out=c2, in0=el8[:, 2:6], in1=el, op=AL.is_gt)
            nc.vector.tensor_add(out=rank, in0=rank, in1=c2)
            nc.vector.tensor_tensor(out=c2, in0=el8[:, 3:7], in1=el, op=AL.is_gt)
            nc.vector.tensor_add(out=rank, in0=rank, in1=c2)
            topm = rtr.tile([128, GS], F32, tag="topm")
            nc.vector.tensor_scalar(out=topm, in0=rank, scalar1=1.5,
                                    scalar2=None, op0=AL.is_le)
            wm = rtr.tile([128, GS], F32, tag="wm")
            nc.vector.tensor_mul(out=wm, in0=ep, in1=topm)
            wsum = rtr.tile([128, 1], F32, tag="wsum")
            nc.vector.reduce_sum(wsum, wm, axis=mybir.AxisListType.X)
            nc.vector.tensor_scalar_add(out=wsum, in0=wsum, scalar1=1e-7)
            wrec = rtr.tile([128, 1], F32, tag="wrec")
            nc.vector.reciprocal(out=wrec, in_=wsum)
            fwn = rtr.tile([128, GS], F32, tag="fwn")
            nc.vector.tensor_scalar_mul(out=fwn, in0=wm, scalar1=wrec)
            nc.vector.tensor_scalar_mul(out=fwn, in0=fwn, scalar1=gw)
            g0 = rtr.tile([128, 1], F32, tag="g0")
            nc.vector.tensor_scalar(out=g0, in0=gidx, scalar1=-1.0, scalar2=1.0,
                                    op0=AL.mult, op1=AL.add)
            dm = dm_all[:, tb, :]
            nc.vector.tensor_scalar_mul(out=dm[:, 0:GS], in0=fwn, scalar1=g0)
            nc.vector.tensor_scalar_mul(out=dm[:, GS:E], in0=fwn, scalar1=gidx)
            nc.vector.reduce_sum(sdm_all[:, tb:tb + 1], _f32(dm),
                                 axis=mybir.AxisListType.X)
            nc.tensor.matmul(load_ps[:, 0:E], ones_col, dm,
                             start=(tb == 0), stop=(tb == TB - 1))
            lsq = rtr.tile([128, G + GS], R32, tag="lsq")
            nc.scalar.square(out=lsq, in_=lt)
            nc.tensor.matmul(load_ps[:, E:E + G + GS], ones_col, lsq,
                             start=(tb == 0), stop=(tb == TB - 1))
            nc.sync.dma_start(out=sdm_d[tb:tb + 1, :], in_=sdm_all[:, tb:tb + 1])

        rp_sb = tmp.tile([1, 16], F32, tag="rp")
        nc.vector.memset(rp_sb, 0.0)
        nc.vector.tensor_copy(out=rp_sb[:, 0:14], in_=load_ps[:, 0:14])
        nc.sync.dma_start(out=io["rpart"], in_=rp_sb)

        # ---------- combo weights (D-sharded per group rank, x0.1) ----------
        c1_ps = mm_tile(256)
        c2_ps = mm_tile(256)
        for hc in range(HC):
            wep = wpool.tile([128, A], R32, tag="wep")
            nc.sync.dma_start(
                out=wep, in_=io["weproj"][hc * 128:(hc + 1) * 128, :])
            wap = wpool.tile([128, A], R32, tag="wap")
            nc.sync.dma_start(
                out=wap, in_=io["waproj"][hc * 128:(hc + 1) * 128, :])
            wcr = wpool.tile([128, 512], R32, tag="wcr")
            nc.sync.dma_start(
                out=wcr, in_=io["wcrhs"][hc * 128:(hc + 1) * 128, :])
            nc.tensor.matmul(c1_ps, wep, wcr[:, 0:256],
                             start=(hc == 0), stop=(hc == HC - 1))
            nc.tensor.matmul(c2_ps, wap, wcr[:, 256:512],
                             start=(hc == 0), stop=(hc == HC - 1))
        for cps, coff, tag in ((c1_ps, OFF_C1, "c1w"), (c2_ps, OFF_C2, "c2w")):
            c_sb = tmp.tile([128, 256], R32, tag=tag, bufs=1, name="c_sb")
            nc.vector.tensor_scalar_mul(out=c_sb, in0=cps, scalar1=0.1)
            nc.sync.dma_start(
                out=cin[coff:coff + 256, :].rearrange("(p x) c -> p (x c)", p=128),
                in_=c_sb)

        # ---------- up/gate -> hidden (feature-major [H, NL]) ----------
        hidden_sb = big.tile([128, HC, NL], R32)
        wupT_r = io["wupT"].rearrange("(c p) h -> p c h", p=128)
        wgateT_r = io["wgateT"].rearrange("(c p) h -> p c h", p=128)
        for hc in range(HC):
            wu = xph.tile([128, DC, 128], R32, tag="wu", bufs=2)
            nc.sync.dma_start(out=wu, in_=wupT_r[:, :, hc * 128:(hc + 1) * 128])
            wg = xph.tile([128, DC, 128], R32, tag="wg", bufs=2)
            nc.sync.dma_start(out=wg, in_=wgateT_r[:, :, hc * 128:(hc + 1) * 128])
            up_ps = mm_tile()
            gt_ps = mm_tile()
            for dc in range(DC):
                nc.tensor.matmul(up_ps, wu[:, dc, :], xT_sb[:, dc, :],
                                 start=(dc == 0), stop=(dc == DC - 1))
            for dc in range(DC):
                nc.tensor.matmul(gt_ps, wg[:, dc, :], xT_sb[:, dc, :],
                                 start=(dc == 0), stop=(dc == DC - 1))
            sg = xph.tile([128, NL], F32, tag="sg", bufs=2)
            nc.scalar.activation(out=sg, in_=gt_ps, func=AF.Silu)
            nc.vector.tensor_mul(out=hidden_sb[:, hc, :], in0=sg, in1=up_ps)

        xph_cm.__exit__(None, None, None)

        # ---------- ao^T = LN(hidden @ W_post.T)^T -> cin ----------
        t1_ps = ps_acc.tile([128, NL], F32, tag="acc")
        for hc in range(HC):
            wpost = wpool.tile([128, A], R32, tag="wpost")
            nc.sync.dma_start(
                out=wpost, in_=io["wpostT"][hc * 128:(hc + 1) * 128, :])
            nc.tensor.matmul(t1_ps, wpost, hidden_sb[:, hc, :],
                             start=(hc == 0), stop=(hc == HC - 1))
        t1_sb = tmp.tile([128, NL], R32, tag="t1sb", bufs=1)
        nc.vector.tensor_copy(out=t1_sb, in_=t1_ps)
        aoT_sb = big.tile([128, NL], R32)
        part_ln(t1_sb, aoT_sb)
        nc.sync.dma_start(
            out=cin[OFF_AO:OFF_AO + ROWS_AO, :].rearrange(
                "(p x) c -> p (x c)", p=128),
            in_=aoT_sb)

        # ---------- the one collective ----------
        nc.gpsimd.collective_compute(
            "AllGather", mybir.AluOpType.bypass,
            replica_groups=[[0, 1, 2, 3], [4, 5, 6, 7]],
            ins=[cin[:].opt()], outs=[cout[:].opt()],
        )

        late = ctx.enter_context(tc.tile_pool(name="late", bufs=1))

        # ---------- down-proj: shared^T ----------
        shared_sb = big.tile([128, DC, NL], F32)
        wdownT_r = io["wdownT"].rearrange("(c p) d -> p c d", p=128)
        for dc in range(DC):
            wd = wpool.tile([128, HC, 128], R32, tag="wd")
            nc.sync.dma_start(out=wd, in_=wdownT_r[:, :, dc * 128:(dc + 1) * 128])
            sh_ps = mm_tile()
            for hc in range(HC):
                nc.tensor.matmul(sh_ps, wd[:, hc, :], hidden_sb[:, hc, :],
                                 start=(hc == 0), stop=(hc == HC - 1))
            nc.vector.tensor_copy(out=shared_sb[:, dc, :], in_=sh_ps)

        # ---------- expert adapters -> hc^T (feature-major) ----------
        hcT_sb = late.tile([128, NL], R32)
        for tb in range(TB):
            he0 = mm_tile()
            he1 = mm_tile()
            nc.tensor.matmul(he0, pre_sb[:, tb * 128:(tb + 1) * 128],
                             aexp_sb[:, 0:512], start=True, stop=True)
            nc.tensor.matmul(he1, pre_sb[:, tb * 128:(tb + 1) * 128],
                             aexp_sb[:, 512:1024], start=True, stop=True)
            he_ln = late.tile([128, E * A], F32, tag="heln", bufs=2)
            for e in range(E):
                src = he0[:, (e % 4) * A:(e % 4 + 1) * A] if e < 4 else \
                    he1[:, (e - 4) * A:(e - 3) * A]
                seg = slice(e * A, (e + 1) * A)
                stats = rtr.tile([128, 6], F32, tag="bnst")
                nc.vector.bn_stats(out=stats, in_=src)
                mv = rtr.tile([128, 2], F32, tag="bnmv")
                nc.vector.bn_aggr(out=mv, in_=stats)
                sd = rtr.tile([128, 1], F32, tag="hesd")
                nc.scalar.activation(out=sd, in_=mv[:, 1:2], func=AF.Sqrt,
                                     bias=epsP, scale=1.0)
                rsd = rtr.tile([128, 1], F32, tag="hers")
                nc.vector.reciprocal(out=rsd, in_=sd)
                nc.vector.tensor_scalar(out=he_ln[:, seg], in0=src,
                                        scalar1=mv[:, 0:1], scalar2=rsd,
                                        op0=AL.subtract, op1=AL.mult)
            nc.vector.tensor_mul(out=he_ln, in0=he_ln, in1=lnge_bc)
            nc.vector.tensor_add(out=he_ln, in0=he_ln, in1=lnbe_bc)
            hc_tok = late.tile([128, A], F32, tag="hctok", bufs=2)
            nc.vector.tensor_scalar_mul(out=hc_tok, in0=he_ln[:, 0:A],
                                        scalar1=_f32(dm_all)[:, tb, 0:1])
            acc = late.tile([128, A], F32, tag="hcacc", bufs=2)
            for e in range(1, E):
                seg = slice(e * A, (e + 1) * A)
                nc.vector.tensor_scalar_mul(out=acc, in0=he_ln[:, seg],
                                            scalar1=_f32(dm_all)[:, tb, e:e + 1])
                nc.vector.tensor_add(out=hc_tok, in0=hc_tok, in1=acc)
            tp = ps_tp.tile([128, 128], F32, tag="tp")
            nc.tensor.transpose(tp, hc_tok, ident)
            nc.vector.tensor_copy(out=hcT_sb[:, tb * 128:(tb + 1) * 128], in_=tp)

        # ---------- gather results ----------
        aoT_all = []
        ai_tok_all = []
        c1_sb = []
        c2_sb = []
        for r in range(GRP):
            base = r * CIN_ROWS
            t_ao = late.tile([128, NL], R32, name=f"aoTall{r}")
            nc.sync.dma_start(
                out=t_ao,
                in_=cout[base + OFF_AO:base + OFF_AO + ROWS_AO, :].rearrange(
                    "(p x) c -> p (x c)", p=128))
            aoT_all.append(t_ao)
            t_ai = late.tile([128, TB, 128], R32, name=f"aitok{r}")
            nc.sync.dma_start(
                out=t_ai,
                in_=cout[base:base + ROWS_AI, :].rearrange(
                    "(t p) c -> p t c", p=128))
            ai_tok_all.append(t_ai)
            t_c1 = late.tile([128, 256], R32, name=f"c1rd{r}")
            nc.sync.dma_start(
                out=t_c1,
                in_=cout[base + OFF_C1:base + OFF_C1 + ROWS_C1, :].rearrange(
                    "(p x) c -> p (x c)", p=128))
            c1_sb.append(t_c1)
            t_c2 = late.tile([128, 256], R32, name=f"c2rd{r}")
            nc.sync.dma_start(
                out=t_c2,
                in_=cout[base + OFF_C2:base + OFF_C2 + ROWS_C2, :].rearrange(
                    "(p x) c -> p (x c)", p=128))
            c2_sb.append(t_c2)

        # ---------- adapter attention ----------
        adapt_ps = ps_acc.tile([128, NL], F32, tag="acc")
        nkb = GRP * TB
        for kb in range(nkb):
            r, j = divmod(kb, TB)
            sc_ps = mm_tile()
            nc.tensor.matmul(sc_ps, aoT_all[r][:, j * 128:(j + 1) * 128],
                             aiT_sb, start=True, stop=True)
            aw = late.tile([128, NL], R32, tag="aw", bufs=3)
            nc.vector.tensor_scalar(out=aw, in0=sc_ps, scalar1=5.0, scalar2=-5.0,
                                    op0=AL.min, op1=AL.max)
            nc.scalar.activation(out=aw, in_=_f32(aw), func=AF.Silu)
            nc.tensor.matmul(adapt_ps, ai_tok_all[r][:, j, :], aw,
                             start=(kb == 0), stop=(kb == nkb - 1))
        adapt_sb = late.tile([128, NL], R32)
        nc.vector.tensor_copy(out=adapt_sb, in_=adapt_ps)

        # ---------- sdm broadcast row (plain f32 matmul) ----------
        sdm_row = late.tile([1, NL], F32, tag="sdmrow")
        nc.sync.dma_start(out=sdm_row,
                          in_=sdm_d[:].rearrange("(o a) b -> o (a b)", o=1))
        sdm_bc_ps = mm_tile()
        nc.tensor.matmul(sdm_bc_ps, ones_row, sdm_row, start=True, stop=True)
        sdm_bc = late.tile([128, NL], F32)
        nc.vector.tensor_copy(out=sdm_bc, in_=sdm_bc_ps)

        # ---------- epilogue ----------
        outT_r = io["outT"].rearrange("(c p) n -> c p n", p=128)
        for dc in range(DC):
            r, half = divmod(dc, 2)
            c1l = c1_sb[r][:, half * 128:(half + 1) * 128]
            c2l = c2_sb[r][:, half * 128:(half + 1) * 128]
            s2_ps = mm_tile()
            nc.tensor.matmul(s2_ps, c2l, adapt_sb, start=True, stop=True)
            moe_ps = mm_tile()
            nc.tensor.matmul(moe_ps, c1l, hcT_sb, start=True, stop=True)
            ot = late.tile([128, NL], F32, tag="ot", bufs=2)
            nc.vector.tensor_add(out=ot, in0=shared_sb[:, dc, :], in1=s2_ps)
            nc.vector.tensor_mul(out=ot, in0=ot, in1=sdm_bc)
            nc.vector.tensor_add(out=ot, in0=ot, in1=moe_ps)
            nc.sync.dma_start(out=outT_r[dc], in_=ot)


_CACHE = {}


def _get_program():
    if "nc" not in _CACHE:
        _CACHE["nc"] = build_program()
    return _CACHE["nc"]


def kernel(x, W_up, W_gate, W_down, W_pre, W_post, ln_g, ln_b, W_aproj,
           A_exp, ln_g_e, ln_b_e, W_eproj, W_oproj, W_rg, W_re):
    nc = _get_program()
    f = np.float32
    xf = np.ascontiguousarray(np.asarray(x, f).reshape(N, D))
    wupT = np.ascontiguousarray(np.asarray(W_up, f).T)      # [D,H]
    wgateT = np.ascontiguousarray(np.asarray(W_gate, f).T)  # [D,H]
    wdownT = np.ascontiguousarray(np.asarray(W_down, f).T)  # [H,D]
    wpreT = np.ascontiguousarray(np.asarray(W_pre, f).T)    # [D,A]
    wpostT = np.ascontiguousarray(np.asarray(W_post, f).T)  # [H,A]
    waproj = np.ascontiguousarray(np.asarray(W_aproj, f))   # [H,A]
    weproj = np.ascontiguousarray(np.asarray(W_eproj, f))   # [H,A]
    woprojT = np.ascontiguousarray(np.asarray(W_oproj, f).T)  # [H,D]
    # aexpP[a, e*A+c] = A_exp[e, c, a]
    aexpP = np.ascontiguousarray(
        np.asarray(A_exp, f).transpose(2, 0, 1).reshape(A, E * A))
    wrge = np.ascontiguousarray(
        np.concatenate([np.asarray(W_rg, f), np.asarray(W_re, f)], axis=0).T)
    lngb = np.ascontiguousarray(
        np.stack([np.asarray(ln_g, f), np.asarray(ln_b, f)], axis=1))  # [A,2]
    lngeP = np.ascontiguousarray(np.asarray(ln_g_e, f).reshape(1, E * A))
    lnbeP = np.ascontiguousarray(np.asarray(ln_b_e, f).reshape(1, E * A))

    in_maps = []
    wcrhs_cache = {}
    for c in range(NC):
        g = c % GRP
        if g not in wcrhs_cache:
            wcrhs_cache[g] = np.ascontiguousarray(np.concatenate(
                [woprojT[:, g * 256:(g + 1) * 256],
                 wdownT[:, g * 256:(g + 1) * 256]], axis=1))
        xT_c = np.ascontiguousarray(xf[c * NL:(c + 1) * NL].T)  # [D, NL]
        in_maps.append({
            "xT": xT_c, "wupT": wupT, "wgateT": wgateT, "wdownT": wdownT,
            "wpreT": wpreT, "wpostT": wpostT, "waproj": waproj,
            "weproj": weproj, "wcrhs": wcrhs_cache[g], "aexpP": aexpP,
            "wrge": wrge, "lngb": lngb, "lngeP": lngeP, "lnbeP": lnbeP,
        })

    res = bass_utils.run_bass_kernel_spmd(nc, in_maps, core_ids=list(range(NC)))

    out = np.empty((N, D), f)
    load = np.zeros(E, np.float64)
    sq = np.zeros(6, np.float64)
    for c in range(NC):
        out[c * NL:(c + 1) * NL] = res.results[c]["outT"].T
        rp = res.results[c]["rpart"][0]
        load += rp[0:E]
        sq += rp[E:E + 6]
    target = load.sum() / E
    router_loss = 0.001 * (np.mean((load - target) ** 2)
                           + sq[0:G].sum() / (N * G)
                           + sq[G:G + GS].sum() / (N * GS))
    return out.reshape(B, S, D), np.float32(router_loss)


# revision 10
# speedup vs baseline: 1.6618x; 1.6618x over previous
"""Trainium2 Bass kernel for nn_MoEEncoderDecoderGPT (moe_routing).

Sharding: 4096 tokens x 512/core over 8 cores (cores 0-3 batch 0, 4-7 batch 1).
One grouped AllGather ([[0-3],[4-7]]) moves ai (token-major), ao^T
(feature-major) and the D-sharded combo weights; router-loss partials are
reduced on host. The per-expert [E,N,H]x[H,D] einsums are collapsed:
  moe_out = (sum_e w_e * LN_e(pre @ A_e^T)) @ (W_oproj @ W_eproj)^T   [combo1]
  shared  = hidden @ W_down^T + 0.1 * adapt @ (W_down @ W_aproj)^T    [combo2]
Big matmuls run in float32r (full PE rate on fp32 data). The BIR verifier
requires fp32r matmul inputs to be *produced* as fp32r, so those tiles are
declared float32r; DVE/ACT reads of them go through a bitcast-to-f32 view.
"""

import contextlib

import numpy as np

import concourse.bass as bass
import concourse.tile as tile_mod
from concourse import bacc
from concourse import mybir
from concourse import bass_utils
from concourse.masks import make_identity
from concourse.vector_clock import ScopedClock

# ---- problem constants --------------------------------------------------
B, S, D = 2, 2048, 1024
E, GS, KTOP = 8, 4, 2
H = 2 * D            # 2048
A = H // 16          # 128
G = E // GS          # 2
N = B * S            # 4096
NC = 8               # cores
NL = N // NC         # 512 tokens per core
DC = D // 128        # 8 d-chunks
HC = H // 128        # 16 h-chunks
TB = NL // 128       # 4 token blocks per core
GRP = 4              # ranks per AllGather group
LN_EPS = 1e-5
R32 = mybir.dt.float32r
F32 = mybir.dt.float32

# cin row layout (width 128 f32): per-rank AllGather contribution
ROWS_AI = NL                      # 512 rows: ai token-major [512,128]
ROWS_AO = NL                      # 512 rows: aoT [128,512] viewed (p x) c
ROWS_C1 = 256                     # combo1T shard [128,256] viewed (p x) c
ROWS_C2 = 256                     # combo2T shard [128,256]
CIN_ROWS = ROWS_AI + ROWS_AO + ROWS_C1 + ROWS_C2   # 1536
OFF_AO = ROWS_AI
OFF_C1 = ROWS_AI + ROWS_AO
OFF_C2 = OFF_C1 + ROWS_C1


# ---- workaround: this walrus build allows only ONE sync-wait per Drain ----
def _patched_drain_and_barrier(self, tick_clock, wait_clock):
    nc = self.nc
    drain_inst = nc.sync.drain()
    wait_clock.add_sem_waits(
        drain_inst.ins, ScopedClock({None: tick_clock.global_clock})
    )
    si = drain_inst.ins.sync_info
    if si is not None and len(si.on_wait) > 1:
        waits = list(si.on_wait)
        ups = list(si.on_update)
        drain_inst.ins.sync_info = mybir.SyncInfo(on_wait=[waits[0]], on_update=[])
        last = drain_inst
        for w in waits[1:]:
            d = nc.sync.drain()
            d.ins.sync_info = mybir.SyncInfo(on_wait=[w], on_update=[])
            last = d
        if ups:
            lsi = last.ins.sync_info
            last.ins.sync_info = mybir.SyncInfo(
                on_wait=list(lsi.on_wait), on_update=ups
            )
    nc.all_engine_barrier()
    assert self.sems is not None
    popped = nc._tile_sem_poison_stack.pop()
    assert popped is self._sem_poison
    nc.clear_and_free_semaphores(list(self.sems.allocated().values()))
    nc.all_engine_barrier()


tile_mod.TileContext._drain_and_barrier = _patched_drain_and_barrier


def _f32(ap):
    """f32 view of a float32r tile for DVE/ACT reads"""
    return ap.bitcast(F32)


def build_program():
    nc = bacc.Bacc("TRN2", target_bir_lowering=False, debug=False,
                   num_devices=NC)
    io = {}
    for name, shape, dt in [
        ("xT", [D, NL], R32), ("wupT", [D, H], R32), ("wgateT", [D, H], R32),
        ("wdownT", [H, D], R32), ("wpreT", [D, A], R32),
        ("wpostT", [H, A], R32), ("waproj", [H, A], R32),
        ("weproj", [H, A], R32), ("wcrhs", [H, 512], R32),
        ("aexpP", [A, E * A], R32), ("wrge", [D, G + GS], R32),
        ("lngb", [A, 2], F32), ("lngeP", [1, E * A], F32),
        ("lnbeP", [1, E * A], F32),
    ]:
        io[name] = nc.dram_tensor(name, shape, dt, kind="ExternalInput").ap()
    io["outT"] = nc.dram_tensor("outT", [D, NL], F32, kind="ExternalOutput").ap()
    io["rpart"] = nc.dram_tensor("rpart", [1, 16], F32,
                                 kind="ExternalOutput").ap()

    with tile_mod.TileContext(nc) as tc:
        _body(nc, tc, io)
    nc.compile()
    return nc


def _body(nc, tc, io):
    AL = mybir.AluOpType
    AF = mybir.ActivationFunctionType

    ctx = contextlib.ExitStack()
    with ctx:
        singles = ctx.enter_context(tc.tile_pool(name="singles", bufs=1))
        big = ctx.enter_context(tc.tile_pool(name="big", bufs=1))
        wpool = ctx.enter_context(tc.tile_pool(name="wpool", bufs=2))
        tmp = ctx.enter_context(tc.tile_pool(name="tmp", bufs=2))
        rtr = ctx.enter_context(tc.tile_pool(name="rtr", bufs=2))
        # PSUM budget: 8 banks = mm(3) + acc(2) + tp(2) + row(1)
        ps_mm = ctx.enter_context(tc.tile_pool(name="ps_mm", bufs=3, space="PSUM"))
        ps_acc = ctx.enter_context(tc.tile_pool(name="ps_acc", bufs=2, space="PSUM"))
        ps_tp = ctx.enter_context(tc.tile_pool(name="ps_tp", bufs=1, space="PSUM"))
        ps_row = ctx.enter_context(tc.tile_pool(name="ps_row", bufs=1, space="PSUM"))
        dram = ctx.enter_context(tc.tile_pool(name="dram", bufs=1, space="DRAM"))
        xph_cm = tc.tile_pool(name="xph", bufs=1)
        xph = xph_cm.__enter__()

        def mm_tile(cols=NL):
            return ps_mm.tile([128, cols], F32, tag="mm", name="mm",
                              padded_shape=[128, NL])

        # ---------- constants / small loads ----------
        ident = singles.tile([128, 128], F32)
        make_identity(nc, ident)
        ones_col = singles.tile([128, 1], R32)
        nc.vector.memset(_f32(ones_col), 1.0)
        ones_row = singles.tile([1, 128], F32)
        nc.vector.memset(ones_row, 1.0)
        eps1 = singles.tile([1, 1], F32)
        nc.vector.memset(eps1, LN_EPS)
        epsP = singles.tile([128, 1], F32)
        nc.vector.memset(epsP, LN_EPS)
        lngb_sb = singles.tile([A, 2], F32)
        nc.gpsimd.dma_start(out=lngb_sb, in_=io["lngb"])
        lnge_bc = singles.tile([128, E * A], F32)
        nc.gpsimd.dma_start(out=lnge_bc, in_=io["lngeP"].to_broadcast([128, E * A]))
        lnbe_bc = singles.tile([128, E * A], F32)
        nc.gpsimd.dma_start(out=lnbe_bc, in_=io["lnbeP"].to_broadcast([128, E * A]))
        wrge_sb = singles.tile([128, DC, G + GS], R32)
        nc.sync.dma_start(out=wrge_sb,
                          in_=io["wrge"].rearrange("(c p) g -> p c g", p=128))
        aexp_sb = singles.tile([128, E * A], R32)
        nc.sync.dma_start(out=aexp_sb, in_=io["aexpP"])
        wpre_sb = singles.tile([128, DC, A], R32)
        nc.sync.dma_start(out=wpre_sb,
                          in_=io["wpreT"].rearrange("(c p) a -> p c a", p=128))

        # xT: [D, NL] -> sbuf [128, DC, NL]
        xT_sb = xph.tile([128, DC, NL], R32)
        nc.sync.dma_start(out=xT_sb,
                          in_=io["xT"].rearrange("(c p) n -> p c n", p=128))

        # DRAM bounce buffers for the collective + sdm row trick
        cin = dram.tile([CIN_ROWS, 128], R32)
        # note: Shared addr space is rejected for 4-rank groups by this stack
        cout = dram.tile([GRP * CIN_ROWS, 128], R32)
        sdm_d = dram.tile([TB, 128], F32)

        # ---------- pre^T = (x @ W_pre.T)^T  (feature-major [A, NL]) ----------
        pre_ps = mm_tile()
        for dc in range(DC):
            nc.tensor.matmul(pre_ps, wpre_sb[:, dc, :], xT_sb[:, dc, :],
                             start=(dc == 0), stop=(dc == DC - 1))
        pre_sb = big.tile([128, NL], R32)
        nc.vector.tensor_copy(out=pre_sb, in_=pre_ps)

        # ---------- partition-major LN (over A=128 partitions) ----------
        def part_ln(src_sb, dst_sb):
            """src_sb/dst_sb are float32r [128, NL]."""
            sums = ps_row.tile([1, NL], F32, tag="row", padded_shape=[1, NL])
            nc.tensor.matmul(sums, ones_col, src_sb, start=True, stop=True)
            sq_sb = tmp.tile([128, NL], R32, tag="lnsq", bufs=1)
            nc.scalar.square(out=sq_sb, in_=_f32(src_sb))
            sqs = ps_row.tile([1, NL], F32, tag="row", padded_shape=[1, NL])
            nc.tensor.matmul(sqs, ones_col, sq_sb, start=True, stop=True)
            pack = tmp.tile([1, 2 * NL], F32, tag="lnpack", bufs=1)
            mu = pack[:, 0:NL]
            rs = pack[:, NL:2 * NL]
            nc.vector.tensor_scalar_mul(out=mu, in0=sums, scalar1=1.0 / 128.0)
            musq = tmp.tile([1, NL], F32, tag="lnmusq", bufs=1)
            nc.scalar.square(out=musq, in_=mu)
            ex2 = tmp.tile([1, NL], F32, tag="lnex2", bufs=1)
            nc.vector.tensor_scalar_mul(out=ex2, in0=sqs, scalar1=1.0 / 128.0)
            var = tmp.tile([1, NL], F32, tag="lnvar", bufs=1)
            nc.vector.tensor_sub(out=var, in0=ex2, in1=musq)
            sd = tmp.tile([1, NL], F32, tag="lnsd", bufs=1)
            nc.scalar.activation(out=sd, in_=var, func=AF.Sqrt,
                                 bias=eps1, scale=1.0)
            nc.vector.reciprocal(out=rs, in_=sd)
            # plain-f32 broadcast matmuls (4 cyc/row, negligible here)
            mu_bc = mm_tile()
            nc.tensor.matmul(mu_bc, ones_row, mu, start=True, stop=True)
            rs_bc = mm_tile()
            nc.tensor.matmul(rs_bc, ones_row, rs, start=True, stop=True)
            t = tmp.tile([128, NL], F32, tag="lnt", bufs=1)
            nc.vector.tensor_sub(out=t, in0=_f32(src_sb), in1=mu_bc)
            nc.vector.tensor_mul(out=t, in0=t, in1=rs_bc)
            nc.vector.tensor_scalar(out=dst_sb, in0=t,
                                    scalar1=lngb_sb[:, 0:1],
                                    scalar2=lngb_sb[:, 1:2],
                                    op0=AL.mult, op1=AL.add)

        # ai^T = LN(pre^T)  (feature-major; scores rhs)
        aiT_sb = big.tile([128, NL], R32)
        part_ln(pre_sb, aiT_sb)

        # ai token-major -> cin rows 0:512
        for tb in range(TB):
            tp = ps_tp.tile([128, 128], F32, tag="tp")
            nc.tensor.transpose(tp, _f32(aiT_sb)[:, tb * 128:(tb + 1) * 128],
                                ident)
            ai_tok = tmp.tile([128, 128], R32, tag="aitok")
            nc.vector.tensor_copy(out=ai_tok, in_=tp)
            nc.sync.dma_start(out=cin[tb * 128:(tb + 1) * 128, :], in_=ai_tok)

        # ---------- router ----------
        lgT_ps = mm_tile()
        for dc in range(DC):
            nc.tensor.matmul(lgT_ps[0:G + GS, :], wrge_sb[:, dc, :],
                             xT_sb[:, dc, :],
                             start=(dc == 0), stop=(dc == DC - 1))
        lgT_sb = xph.tile([G + GS, NL], F32, tag="lgT")
        nc.vector.tensor_copy(out=lgT_sb, in_=lgT_ps[0:G + GS, :])

        dm_all = big.tile([128, TB, E], R32)
        sdm_all = big.tile([128, TB], F32)
        load_ps = ps_row.tile([1, 16], F32, tag="row", padded_shape=[1, NL])
        lsq_ps = ps_row.tile([1, 8], F32, tag="row2", padded_shape=[1, NL])
        for tb in range(TB):
            ltp = ps_tp.tile([128, 128], F32, tag="tp")
            nc.tensor.transpose(ltp[:, 0:G + GS],
                                lgT_sb[:, tb * 128:(tb + 1) * 128],
                                ident[0:G + GS, 0:G + GS])
            lt = rtr.tile([128, G + GS], F32, tag="lt")
            nc.vector.tensor_copy(out=lt, in_=ltp[:, 0:G + GS])
            gl0, gl1, el = lt[:, 0:1], lt[:, 1:2], lt[:, 2:6]
            gdiff = rtr.tile([128, 1], F32, tag="gdiff")
            nc.vector.tensor_sub(out=gdiff, in0=gl1, in1=gl0)
            gidx = rtr.tile([128, 1], F32, tag="gidx")
            nc.vector.tensor_scalar(out=gidx, in0=gdiff, scalar1=0.0,
                                    scalar2=None, op0=AL.is_gt)
            gabs = rtr.tile([128, 1], F32, tag="gabs")
            nc.scalar.activation(out=gabs, in_=gdiff, func=AF.Abs)
            gw = rtr.tile([128, 1], F32, tag="gw")
            nc.scalar.activation(out=gw, in_=gabs, func=AF.Sigmoid)
            m4 = rtr.tile([128, 1], F32, tag="m4")
            nc.vector.reduce_max(m4, el, axis=mybir.AxisListType.X)
            negm = rtr.tile([128, 1], F32, tag="negm")
            nc.vector.tensor_scalar_mul(out=negm, in0=m4, scalar1=-1.0)
            eexp = rtr.tile([128, GS], F32, tag="eexp")
            nc.scalar.activation(out=eexp, in_=el, func=AF.Exp,
                                 bias=negm, scale=1.0)
            ssum = rtr.tile([128, 1], F32, tag="ssum")
            nc.vector.reduce_sum(ssum, eexp, axis=mybir.AxisListType.X)
            srec = rtr.tile([128, 1], F32, tag="srec")
            nc.vector.reciprocal(out=srec, in_=ssum)
            ep = rtr.tile([128, GS], F32, tag="ep")
            nc.vector.tensor_scalar_mul(out=ep, in0=eexp, scalar1=srec)
            el8 = rtr.tile([128, 2 * GS], F32, tag="el8")
            nc.vector.tensor_copy(out=el8[:, 0:GS], in_=el)
            nc.vector.tensor_copy(out=el8[:, GS:2 * GS], in_=el)
            rank = rtr.tile([128, GS], F32, tag="rank")
            c2 = rtr.tile([128, GS], F32, tag="c2")
            nc.vector.tensor_tensor(out=rank, in0=el8[:, 1:5], in1=el, op=AL.is_gt)
            nc.vector.tensor_tensor(out=c2, in0=el8[:, 2:6], in1=el, op=AL.is_gt)
            nc.vector.tensor_add(out=rank, in0=rank, in1=c2)
            nc.vector.tensor_tensor(out=c2, in0=el8[:, 3:7], in1=el, op=AL.is_gt)
            nc.vector.tensor_add(out=rank, in0=rank, in1=c2)
            topm = rtr.tile([128, GS], F32, tag="topm")
            nc.vector.tensor_scalar(out=topm, in0=rank, scalar1=1.5,
                                    scalar2=None, op0=AL.is_le)
            wm = rtr.tile([128, GS], F32, tag="wm")
            nc.vector.tensor_mul(out=wm, in0=ep, in1=topm)
            wsum = rtr.tile([128, 1], F32, tag="wsum")
            nc.vector.reduce_sum(wsum, wm, axis=mybir.AxisListType.X)
            nc.vector.tensor_scalar_add(out=wsum, in0=wsum, scalar1=1e-7)
            wrec = rtr.tile([128, 1], F32, tag="wrec")
            nc.vector.reciprocal(out=wrec, in_=wsum)
            fwn = rtr.tile([128, GS], F32, tag="fwn")
            nc.vector.tensor_scalar_mul(out=fwn, in0=wm, scalar1=wrec)
            nc.vector.tensor_scalar_mul(out=fwn, in0=fwn, scalar1=gw)
            g0 = rtr.tile([128, 1], F32, tag="g0")
            nc.vector.tensor_scalar(out=g0, in0=gidx, scalar1=-1.0, scalar2=1.0,
                                    op0=AL.mult, op1=AL.add)
            dm = dm_all[:, tb, :]
            nc.vector.tensor_scalar_mul(out=dm[:, 0:GS], in0=fwn, scalar1=g0)
            nc.vector.tensor_scalar_mul(out=dm[:, GS:E], in0=fwn, scalar1=gidx)
            nc.vector.reduce_sum(sdm_all[:, tb:tb + 1], _f32(dm),
                                 axis=mybir.AxisListType.X)
            nc.tensor.matmul(load_ps[:, 0:E], ones_col, dm,
                             start=(tb == 0), stop=(tb == TB - 1))
            lsq = rtr.tile([128, G + GS], R32, tag="lsq")
            nc.scalar.square(out=lsq, in_=lt)
            nc.tensor.matmul(lsq_ps[:, 0:G + GS], ones_col, lsq,
                             start=(tb == 0), stop=(tb == TB - 1))
            nc.sync.dma_start(out=sdm_d[tb:tb + 1, :], in_=sdm_all[:, tb:tb + 1])

        rp_sb = tmp.tile([1, 16], F32, tag="rp")
        nc.vector.memset(rp_sb, 0.0)
        nc.vector.tensor_copy(out=rp_sb[:, 0:E], in_=load_ps[:, 0:E])
        nc.vector.tensor_copy(out=rp_sb[:, E:E + G + GS], in_=lsq_ps[:, 0:G + GS])
        nc.sync.dma_start(out=io["rpart"], in_=rp_sb)

        # ---------- combo weights (D-sharded per group rank, x0.1) ----------
        c1_ps = mm_tile(256)
        c2_ps = mm_tile(256)
        for hc in range(HC):
            wep = wpool.tile([128, A], R32, tag="wep")
            nc.sync.dma_start(
                out=wep, in_=io["weproj"][hc * 128:(hc + 1) * 128, :])
            wap = wpool.tile([128, A], R32, tag="wap")
            nc.sync.dma_start(
                out=wap, in_=io["waproj"][hc * 128:(hc + 1) * 128, :])
            wcr = wpool.tile([128, 512], R32, tag="wcr")
            nc.sync.dma_start(
                out=wcr, in_=io["wcrhs"][hc * 128:(hc + 1) * 128, :])
            nc.tensor.matmul(c1_ps, wep, wcr[:, 0:256],
                             start=(hc == 0), stop=(hc == HC - 1))
            nc.tensor.matmul(c2_ps, wap, wcr[:, 256:512],
                             start=(hc == 0), stop=(hc == HC - 1))
        for cps, coff, tag in ((c1_ps, OFF_C1, "c1w"), (c2_ps, OFF_C2, "c2w")):
            c_sb = tmp.tile([128, 256], R32, tag=tag, bufs=1, name="c_sb")
            nc.vector.tensor_scalar_mul(out=c_sb, in0=cps, scalar1=0.1)
            nc.sync.dma_start(
                out=cin[coff:coff + 256, :].rearrange("(p x) c -> p (x c)", p=128),
                in_=c_sb)

        # ---------- up/gate -> hidden (feature-major [H, NL]) ----------
        hidden_sb = big.tile([128, HC, NL], R32)
        wupT_r = io["wupT"].rearrange("(c p) h -> p c h", p=128)
        wgateT_r = io["wgateT"].rearrange("(c p) h -> p c h", p=128)
        for hc in range(HC):
            wu = xph.tile([128, DC, 128], R32, tag="wu", bufs=2)
            nc.sync.dma_start(out=wu, in_=wupT_r[:, :, hc * 128:(hc + 1) * 128])
            wg = xph.tile([128, DC, 128], R32, tag="wg", bufs=2)
            nc.sync.dma_start(out=wg, in_=wgateT_r[:, :, hc * 128:(hc + 1) * 128])
            up_ps = mm_tile()
            gt_ps = mm_tile()
            for dc in range(DC):
                nc.tensor.matmul(up_ps, wu[:, dc, :], xT_sb[:, dc, :],
                                 start=(dc == 0), stop=(dc == DC - 1))
            for dc in range(DC):
                nc.tensor.matmul(gt_ps, wg[:, dc, :], xT_sb[:, dc, :],
                                 start=(dc == 0), stop=(dc == DC - 1))
            sg = xph.tile([128, NL], F32, tag="sg", bufs=2)
            nc.scalar.activation(out=sg, in_=gt_ps, func=AF.Silu)
            nc.vector.tensor_mul(out=hidden_sb[:, hc, :], in0=sg, in1=up_ps)

        xph_cm.__exit__(None, None, None)

        # ---------- ao^T = LN(hidden @ W_post.T)^T -> cin ----------
        t1_ps = ps_acc.tile([128, NL], F32, tag="acc")
        for hc in range(HC):
            wpost = wpool.tile([128, A], R32, tag="wpost")
            nc.sync.dma_start(
                out=wpost, in_=io["wpostT"][hc * 128:(hc + 1) * 128, :])
            nc.tensor.matmul(t1_ps, wpost, hidden_sb[:, hc, :],
                             start=(hc == 0), stop=(hc == HC - 1))
        t1_sb = tmp.tile([128, NL], R32, tag="t1sb", bufs=1)
        nc.vector.tensor_copy(out=t1_sb, in_=t1_ps)
        aoT_sb = big.tile([128, NL], R32)
        part_ln(t1_sb, aoT_sb)
        nc.sync.dma_start(
            out=cin[OFF_AO:OFF_AO + ROWS_AO, :].rearrange(
                "(p x) c -> p (x c)", p=128),
            in_=aoT_sb)

        # ---------- the one collective ----------
        nc.gpsimd.collective_compute(
            "AllGather", mybir.AluOpType.bypass,
            replica_groups=[[0, 1, 2, 3], [4, 5, 6, 7]],
            ins=[cin[:].opt()], outs=[cout[:].opt()],
        )

        late = ctx.enter_context(tc.tile_pool(name="late", bufs=1))

        # ---------- down-proj: shared^T ----------
        shared_sb = big.tile([128, DC, NL], F32)
        wdownT_r = io["wdownT"].rearrange("(c p) d -> p c d", p=128)
        for dc in range(DC):
            wd = wpool.tile([128, HC, 128], R32, tag="wd")
            nc.sync.dma_start(out=wd, in_=wdownT_r[:, :, dc * 128:(dc + 1) * 128])
            sh_ps = mm_tile()
            for hc in range(HC):
                nc.tensor.matmul(sh_ps, wd[:, hc, :], hidden_sb[:, hc, :],
                                 start=(hc == 0), stop=(hc == HC - 1))
            nc.vector.tensor_copy(out=shared_sb[:, dc, :], in_=sh_ps)

        # ---------- expert adapters -> hc^T (feature-major) ----------
        hcT_sb = late.tile([128, NL], R32)
        for tb in range(TB):
            he0 = mm_tile()
            he1 = mm_tile()
            nc.tensor.matmul(he0, pre_sb[:, tb * 128:(tb + 1) * 128],
                             aexp_sb[:, 0:512], start=True, stop=True)
            nc.tensor.matmul(he1, pre_sb[:, tb * 128:(tb + 1) * 128],
                             aexp_sb[:, 512:1024], start=True, stop=True)
            he_ln = late.tile([128, E * A], F32, tag="heln", bufs=2)
            for e in range(E):
                src = he0[:, (e % 4) * A:(e % 4 + 1) * A] if e < 4 else \
                    he1[:, (e - 4) * A:(e - 3) * A]
                seg = slice(e * A, (e + 1) * A)
                stats = rtr.tile([128, 6], F32, tag="bnst")
                nc.vector.bn_stats(out=stats, in_=src)
                mv = rtr.tile([128, 2], F32, tag="bnmv")
                nc.vector.bn_aggr(out=mv, in_=stats)
                sd = rtr.tile([128, 1], F32, tag="hesd")
                nc.scalar.activation(out=sd, in_=mv[:, 1:2], func=AF.Sqrt,
                                     bias=epsP, scale=1.0)
                rsd = rtr.tile([128, 1], F32, tag="hers")
                nc.vector.reciprocal(out=rsd, in_=sd)
                nc.vector.tensor_scalar(out=he_ln[:, seg], in0=src,
                                        scalar1=mv[:, 0:1], scalar2=rsd,
                                        op0=AL.subtract, op1=AL.mult)
            nc.vector.tensor_mul(out=he_ln, in0=he_ln, in1=lnge_bc)
            nc.vector.tensor_add(out=he_ln, in0=he_ln, in1=lnbe_bc)
            hc_tok = late.tile([128, A], F32, tag="hctok", bufs=2)
            nc.vector.tensor_scalar_mul(out=hc_tok, in0=he_ln[:, 0:A],
                                        scalar1=_f32(dm_all)[:, tb, 0:1])
            acc = late.tile([128, A], F32, tag="hcacc", bufs=2)
            for e in range(1, E):
                seg = slice(e * A, (e + 1) * A)
                nc.vector.tensor_scalar_mul(out=acc, in0=he_ln[:, seg],
                                            scalar1=_f32(dm_all)[:, tb, e:e + 1])
                nc.vector.tensor_add(out=hc_tok, in0=hc_tok, in1=acc)
            tp = ps_tp.tile([128, 128], F32, tag="tp")
            nc.tensor.transpose(tp, hc_tok, ident)
            nc.vector.tensor_copy(out=hcT_sb[:, tb * 128:(tb + 1) * 128], in_=tp)

        # ---------- gather results ----------
        aoT_all = []
        ai_tok_all = []
        c1_sb = []
        c2_sb = []
        for r in range(GRP):
            base = r * CIN_ROWS
            t_ao = late.tile([128, NL], R32, name=f"aoTall{r}")
            nc.sync.dma_start(
                out=t_ao,
                in_=cout[base + OFF_AO:base + OFF_AO + ROWS_AO, :].rearrange(
                    "(p x) c -> p (x c)", p=128))
            aoT_all.append(t_ao)
            t_ai = late.tile([128, TB, 128], R32, name=f"aitok{r}")
            nc.sync.dma_start(
                out=t_ai,
                in_=cout[base:base + ROWS_AI, :].rearrange(
                    "(t p) c -> p t c", p=128))
            ai_tok_all.append(t_ai)
            t_c1 = late.tile([128, 256], R32, name=f"c1rd{r}")
            nc.sync.dma_start(
                out=t_c1,
                in_=cout[base + OFF_C1:base + OFF_C1 + ROWS_C1, :].rearrange(
                    "(p x) c -> p (x c)", p=128))
            c1_sb.append(t_c1)
            t_c2 = late.tile([128, 256], R32, name=f"c2rd{r}")
            nc.sync.dma_start(
                out=t_c2,
                in_=cout[base + OFF_C2:base + OFF_C2 + ROWS_C2, :].rearrange(
                    "(p x) c -> p (x c)", p=128))
            c2_sb.append(t_c2)

        # ---------- adapter attention ----------
        adapt_ps = ps_acc.tile([128, NL], F32, tag="acc")
        nkb = GRP * TB
        for kb in range(nkb):
            r, j = divmod(kb, TB)
            sc_ps = mm_tile()
            nc.tensor.matmul(sc_ps, aoT_all[r][:, j * 128:(j + 1) * 128],
                             aiT_sb, start=True, stop=True)
            aw = late.tile([128, NL], R32, tag="aw", bufs=3)
            nc.vector.tensor_scalar(out=aw, in0=sc_ps, scalar1=5.0, scalar2=-5.0,
                                    op0=AL.min, op1=AL.max)
            nc.scalar.activation(out=aw, in_=_f32(aw), func=AF.Silu)
            nc.tensor.matmul(adapt_ps, ai_tok_all[r][:, j, :], aw,
                             start=(kb == 0), stop=(kb == nkb - 1))
        adapt_sb = late.tile([128, NL], R32)
        nc.vector.tensor_copy(out=adapt_sb, in_=adapt_ps)

        # ---------- sdm broadcast row (plain f32 matmul) ----------
        sdm_row = late.tile([1, NL], F32, tag="sdmrow")
        nc.sync.dma_start(out=sdm_row,
                          in_=sdm_d[:].rearrange("(o a) b -> o (a b)", o=1))
        sdm_bc_ps = mm_tile()
        nc.tensor.matmul(sdm_bc_ps, ones_row, sdm_row, start=True, stop=True)
        sdm_bc = late.tile([128, NL], F32)
        nc.vector.tensor_copy(out=sdm_bc, in_=sdm_bc_ps)

        # ---------- epilogue ----------
        outT_r = io["outT"].rearrange("(c p) n -> c p n", p=128)
        for dc in range(DC):
            r, half = divmod(dc, 2)
            c1l = c1_sb[r][:, half * 128:(half + 1) * 128]
            c2l = c2_sb[r][:, half * 128:(half + 1) * 128]
            s2_ps = mm_tile()
            nc.tensor.matmul(s2_ps, c2l, adapt_sb, start=True, stop=True)
            moe_ps = mm_tile()
            nc.tensor.matmul(moe_ps, c1l, hcT_sb, start=True, stop=True)
            ot = late.tile([128, NL], F32, tag="ot", bufs=2)
            nc.vector.tensor_add(out=ot, in0=shared_sb[:, dc, :], in1=s2_ps)
            nc.vector.tensor_mul(out=ot, in0=ot, in1=sdm_bc)
            nc.vector.tensor_add(out=ot, in0=ot, in1=moe_ps)
            nc.sync.dma_start(out=outT_r[dc], in_=ot)


_CACHE = {}


def _get_program():
    if "nc" not in _CACHE:
        _CACHE["nc"] = build_program()
    return _CACHE["nc"]


def kernel(x, W_up, W_gate, W_down, W_pre, W_post, ln_g, ln_b, W_aproj,
           A_exp, ln_g_e, ln_b_e, W_eproj, W_oproj, W_rg, W_re):
    nc = _get_program()
    f = np.float32
    xf = np.ascontiguousarray(np.asarray(x, f).reshape(N, D))
    wupT = np.ascontiguousarray(np.asarray(W_up, f).T)      # [D,H]
    wgateT = np.ascontiguousarray(np.asarray(W_gate, f).T)  # [D,H]
    wdownT = np.ascontiguousarray(np.asarray(W_down, f).T)  # [H,D]
    wpreT = np.ascontiguousarray(np.asarray(W_pre, f).T)    # [D,A]
    wpostT = np.ascontiguousarray(np.asarray(W_post, f).T)  # [H,A]
    waproj = np.ascontiguousarray(np.asarray(W_aproj, f))   # [H,A]
    weproj = np.ascontiguousarray(np.asarray(W_eproj, f))   # [H,A]
    woprojT = np.ascontiguousarray(np.asarray(W_oproj, f).T)  # [H,D]
    # aexpP[a, e*A+c] = A_exp[e, c, a]
    aexpP = np.ascontiguousarray(
        np.asarray(A_exp, f).transpose(2, 0, 1).reshape(A, E * A))
    wrge = np.ascontiguousarray(
        np.concatenate([np.asarray(W_rg, f), np.asarray(W_re, f)], axis=0).T)
    lngb = np.ascontiguousarray(
        np.stack([np.asarray(ln_g, f), np.asarray(ln_b, f)], axis=1))  # [A,2]
    lngeP = np.ascontiguousarray(np.asarray(ln_g_e, f).reshape(1, E * A))
    lnbeP = np.ascontiguousarray(np.asarray(ln_b_e, f).reshape(1, E * A))

    in_maps = []
    wcrhs_cache = {}
    for c in range(NC):
        g = c % GRP
        if g not in wcrhs_cache:
            wcrhs_cache[g] = np.ascontiguousarray(np.concatenate(
                [woprojT[:, g * 256:(g + 1) * 256],
                 wdownT[:, g * 256:(g + 1) * 256]], axis=1))
        xT_c = np.ascontiguousarray(xf[c * NL:(c + 1) * NL].T)  # [D, NL]
        in_maps.append({
            "xT": xT_c, "wupT": wupT, "wgateT": wgateT, "wdownT": wdownT,
            "wpreT": wpreT, "wpostT": wpostT, "waproj": waproj,
            "weproj": weproj, "wcrhs": wcrhs_cache[g], "aexpP": aexpP,
            "wrge": wrge, "lngb": lngb, "lngeP": lngeP, "lnbeP": lnbeP,
        })

    res = bass_utils.run_bass_kernel_spmd(nc, in_maps, core_ids=list(range(NC)))

    out = np.empty((N, D), f)
    load = np.zeros(E, np.float64)
    sq = np.zeros(6, np.float64)
    for c in range(NC):
        out[c * NL:(c + 1) * NL] = res.results[c]["outT"].T
        rp = res.results[c]["rpart"][0]
        load += rp[0:E]
        sq += rp[E:E + 6]
    target = load.sum() / E
    router_loss = 0.001 * (np.mean((load - target) ** 2)
                           + sq[0:G].sum() / (N * G)
                           + sq[G:G + GS].sum() / (N * GS))
    return out.reshape(B, S, D), np.float32(router_loss)


# revision 16
# speedup vs baseline: 4.1487x; 2.4965x over previous
"""Trainium2 Bass kernel for nn_MoEEncoderDecoderGPT (moe_routing).

Sharding: 4096 tokens x 512/core over 8 cores (cores 0-3 batch 0, 4-7 batch 1).
One grouped AllGather ([[0-3],[4-7]]) moves ai (token-major), ao^T
(feature-major) and the D-sharded combo weights; router-loss partials are
reduced on host. The per-expert [E,N,H]x[H,D] einsums are collapsed:
  moe_out = (sum_e w_e * LN_e(pre @ A_e^T)) @ (W_oproj @ W_eproj)^T   [combo1]
  shared  = hidden @ W_down^T + 0.1 * adapt @ (W_down @ W_aproj)^T    [combo2]
Big matmuls run in float32r (full PE rate on fp32 data). The BIR verifier
requires fp32r matmul inputs to be *produced* as fp32r, so those tiles are
declared float32r; DVE/ACT reads of them go through a bitcast-to-f32 view.
"""

import contextlib

import numpy as np
import ml_dtypes

import concourse.bass as bass
import concourse.tile as tile_mod
from concourse import bacc
from concourse import mybir
from concourse import bass_utils
from concourse.masks import make_identity
from concourse.vector_clock import ScopedClock

# ---- problem constants --------------------------------------------------
B, S, D = 2, 2048, 1024
E, GS, KTOP = 8, 4, 2
H = 2 * D            # 2048
A = H // 16          # 128
G = E // GS          # 2
N = B * S            # 4096
NC = 8               # cores
NL = N // NC         # 512 tokens per core
DC = D // 128        # 8 d-chunks
HC = H // 128        # 16 h-chunks
TB = NL // 128       # 4 token blocks per core
GRP = 4              # ranks per AllGather group
LN_EPS = 1e-5
R32 = mybir.dt.float32r
F32 = mybir.dt.float32

# cin row layout (width 128 f32): per-rank AllGather contribution
ROWS_AI = NL                      # 512 rows: ai token-major [512,128]
ROWS_AO = NL                      # 512 rows: aoT [128,512] viewed (p x) c
ROWS_C1 = 256                     # combo1T shard [128,256] viewed (p x) c
ROWS_C2 = 256                     # combo2T shard [128,256]
CIN_ROWS = ROWS_AI + ROWS_AO + ROWS_C1 + ROWS_C2   # 1536
OFF_AO = ROWS_AI
OFF_C1 = ROWS_AI + ROWS_AO
OFF_C2 = OFF_C1 + ROWS_C1


# ---- workaround: this walrus build allows only ONE sync-wait per Drain ----
def _patched_drain_and_barrier(self, tick_clock, wait_clock):
    nc = self.nc
    drain_inst = nc.sync.drain()
    wait_clock.add_sem_waits(
        drain_inst.ins, ScopedClock({None: tick_clock.global_clock})
    )
    si = drain_inst.ins.sync_info
    if si is not None and len(si.on_wait) > 1:
        waits = list(si.on_wait)
        ups = list(si.on_update)
        drain_inst.ins.sync_info = mybir.SyncInfo(on_wait=[waits[0]], on_update=[])
        last = drain_inst
        for w in waits[1:]:
            d = nc.sync.drain()
            d.ins.sync_info = mybir.SyncInfo(on_wait=[w], on_update=[])
            last = d
        if ups:
            lsi = last.ins.sync_info
            last.ins.sync_info = mybir.SyncInfo(
                on_wait=list(lsi.on_wait), on_update=ups
            )
    nc.all_engine_barrier()
    assert self.sems is not None
    popped = nc._tile_sem_poison_stack.pop()
    assert popped is self._sem_poison
    nc.clear_and_free_semaphores(list(self.sems.allocated().values()))
    nc.all_engine_barrier()


tile_mod.TileContext._drain_and_barrier = _patched_drain_and_barrier


def _f32(ap):
    """f32 view of a float32r tile for DVE/ACT reads"""
    return ap.bitcast(F32)


def build_program(single=False):
    """single=True: 1-core timing variant for TimelineSim (collective
    replaced by local DRAM copies)."""
    nc = bacc.Bacc("TRN2", target_bir_lowering=False, debug=False,
                   num_devices=1 if single else NC)
    io = {}
    for name, shape, dt in [
        ("xT", [D, NL], R32), ("wupT", [D, H], R32), ("wgateT", [D, H], R32),
        ("wdownT", [H, D], R32), ("wpreT", [D, A], R32),
        ("wpostT", [H, A], R32), ("waproj", [H, A], mybir.dt.bfloat16),
        ("weproj", [H, A], mybir.dt.bfloat16),
        ("wcrhs", [H, 512], mybir.dt.bfloat16),
        ("aexpP", [A, E * A], R32), ("wrge", [D, G + GS], R32),
        ("lngb", [A, 2], F32), ("lngeP", [1, E * A], F32),
        ("lnbeP", [1, E * A], F32),
    ]:
        io[name] = nc.dram_tensor(name, shape, dt, kind="ExternalInput").ap()
    io["outT"] = nc.dram_tensor("outT", [D, NL], F32, kind="ExternalOutput").ap()
    io["rpart"] = nc.dram_tensor("rpart", [1, 16], F32,
                                 kind="ExternalOutput").ap()

    with tile_mod.TileContext(nc) as tc:
        _body(nc, tc, io, single=single)
    nc.compile()
    return nc


def _body(nc, tc, io, single=False):
    AL = mybir.AluOpType
    AF = mybir.ActivationFunctionType

    ctx = contextlib.ExitStack()
    with ctx:
        singles = ctx.enter_context(tc.tile_pool(name="singles", bufs=1))
        big = ctx.enter_context(tc.tile_pool(name="big", bufs=1))
        wpool = ctx.enter_context(tc.tile_pool(name="wpool", bufs=2))
        tmp = ctx.enter_context(tc.tile_pool(name="tmp", bufs=2))
        rtr = ctx.enter_context(tc.tile_pool(name="rtr", bufs=2))
        # PSUM budget: 8 banks = mm(3) + acc(2) + tp(2) + row(1)
        ps_mm = ctx.enter_context(tc.tile_pool(name="ps_mm", bufs=3, space="PSUM"))
        ps_acc = ctx.enter_context(tc.tile_pool(name="ps_acc", bufs=2, space="PSUM"))
        ps_tp = ctx.enter_context(tc.tile_pool(name="ps_tp", bufs=1, space="PSUM"))
        ps_row = ctx.enter_context(tc.tile_pool(name="ps_row", bufs=1, space="PSUM"))
        dram = ctx.enter_context(tc.tile_pool(name="dram", bufs=1, space="DRAM"))
        xph_cm = tc.tile_pool(name="xph", bufs=1)
        xph = xph_cm.__enter__()

        def mm_tile(cols=NL):
            return ps_mm.tile([128, cols], F32, tag="mm", name="mm",
                              padded_shape=[128, NL])

        # ---------- constants / small loads ----------
        ident = singles.tile([128, 128], F32)
        make_identity(nc, ident)
        ones_col = singles.tile([128, 1], R32)
        nc.vector.memset(_f32(ones_col), 1.0)
        ones_row = singles.tile([1, 128], F32)
        nc.vector.memset(ones_row, 1.0)
        eps1 = singles.tile([1, 1], F32)
        nc.vector.memset(eps1, LN_EPS)
        epsP = singles.tile([128, 1], F32)
        nc.vector.memset(epsP, LN_EPS)
        lngb_sb = singles.tile([A, 2], F32)
        nc.gpsimd.dma_start(out=lngb_sb, in_=io["lngb"])
        lnge_bc = singles.tile([128, E * A], F32)
        nc.gpsimd.dma_start(out=lnge_bc, in_=io["lngeP"].to_broadcast([128, E * A]))
        lnbe_bc = singles.tile([128, E * A], F32)
        nc.gpsimd.dma_start(out=lnbe_bc, in_=io["lnbeP"].to_broadcast([128, E * A]))
        wrge_sb = singles.tile([128, DC, G + GS], R32)
        nc.sync.dma_start(out=wrge_sb,
                          in_=io["wrge"].rearrange("(c p) g -> p c g", p=128))


        # xT: [D, NL] -> sbuf [128, DC, NL]
        wpre_sb = singles.tile([128, DC, A], R32)
        wpre_r = io["wpreT"].rearrange("(c p) a -> p c a", p=128)
        xT_sb = xph.tile([128, DC, NL], R32)
        xT_r = io["xT"].rearrange("(c p) n -> p c n", p=128)
        for dc in range(DC):
            nc.sync.dma_start(out=wpre_sb[:, dc, :], in_=wpre_r[:, dc, :])
            nc.sync.dma_start(out=xT_sb[:, dc, :], in_=xT_r[:, dc, :])

        # DRAM bounce buffers for the collective + sdm row trick
        cin = dram.tile([CIN_ROWS, 128], R32)
        # note: Shared addr space is rejected for 4-rank groups by this stack
        cout = dram.tile([GRP * CIN_ROWS, 128], R32)
        sdm_d = dram.tile([TB, 128], F32)

        # ---------- pre^T = (x @ W_pre.T)^T  (feature-major [A, NL]) ----------
        pre_ps = mm_tile()
        for dc in range(DC):
            nc.tensor.matmul(pre_ps, wpre_sb[:, dc, :], xT_sb[:, dc, :],
                             start=(dc == 0), stop=(dc == DC - 1))
        pre_sb = big.tile([128, NL], R32)
        nc.vector.tensor_copy(out=pre_sb, in_=pre_ps)

        # ---------- partition-major LN (over A=128 partitions) ----------
        def part_ln(src_sb, dst_sb):
            """src_sb/dst_sb are float32r [128, NL]."""
            sums = ps_row.tile([1, NL], F32, tag="row", padded_shape=[1, NL])
            nc.tensor.matmul(sums, ones_col, src_sb, start=True, stop=True)
            sq_sb = tmp.tile([128, NL], R32, tag="lnsq", bufs=1)
            nc.scalar.square(out=sq_sb, in_=_f32(src_sb))
            sqs = ps_row.tile([1, NL], F32, tag="row", padded_shape=[1, NL])
            nc.tensor.matmul(sqs, ones_col, sq_sb, start=True, stop=True)
            pack = tmp.tile([1, 2 * NL], F32, tag="lnpack", bufs=1)
            mu = pack[:, 0:NL]
            rs = pack[:, NL:2 * NL]
            nc.vector.tensor_scalar_mul(out=mu, in0=sums, scalar1=1.0 / 128.0)
            musq = tmp.tile([1, NL], F32, tag="lnmusq", bufs=1)
            nc.scalar.square(out=musq, in_=mu)
            ex2 = tmp.tile([1, NL], F32, tag="lnex2", bufs=1)
            nc.vector.tensor_scalar_mul(out=ex2, in0=sqs, scalar1=1.0 / 128.0)
            var = tmp.tile([1, NL], F32, tag="lnvar", bufs=1)
            nc.vector.tensor_sub(out=var, in0=ex2, in1=musq)
            sd = tmp.tile([1, NL], F32, tag="lnsd", bufs=1)
            nc.scalar.activation(out=sd, in_=var, func=AF.Sqrt,
                                 bias=eps1, scale=1.0)
            nc.vector.reciprocal(out=rs, in_=sd)
            # plain-f32 broadcast matmuls (4 cyc/row, negligible here)
            mu_bc = mm_tile()
            nc.tensor.matmul(mu_bc, ones_row, mu, start=True, stop=True)
            rs_bc = mm_tile()
            nc.tensor.matmul(rs_bc, ones_row, rs, start=True, stop=True)
            t = tmp.tile([128, NL], F32, tag="lnt", bufs=1)
            nc.vector.tensor_sub(out=t, in0=_f32(src_sb), in1=mu_bc)
            nc.vector.tensor_mul(out=t, in0=t, in1=rs_bc)
            nc.vector.tensor_scalar(out=dst_sb, in0=t,
                                    scalar1=lngb_sb[:, 0:1],
                                    scalar2=lngb_sb[:, 1:2],
                                    op0=AL.mult, op1=AL.add)

        # ---------- router ----------
        lgT_ps = mm_tile()
        for dc in range(DC):
            nc.tensor.matmul(lgT_ps[0:G + GS, :], wrge_sb[:, dc, :],
                             xT_sb[:, dc, :],
                             start=(dc == 0), stop=(dc == DC - 1))
        lgT_sb = xph.tile([G + GS, NL], F32, tag="lgT")
        nc.vector.tensor_copy(out=lgT_sb, in_=lgT_ps[0:G + GS, :])

        dm_all = big.tile([128, TB, E], R32)
        sdm_all = big.tile([128, TB], F32)
        load_ps = ps_row.tile([1, 16], F32, tag="row", padded_shape=[1, NL])
        lsq_ps = ps_row.tile([1, 8], F32, tag="row2",
                             padded_shape=[128, NL])
        for tb in range(TB):
            ltp = ps_tp.tile([128, 128], F32, tag="tp")
            nc.tensor.transpose(ltp[:, 0:G + GS],
                                lgT_sb[:, tb * 128:(tb + 1) * 128],
                                ident[0:G + GS, 0:G + GS])
            lt = rtr.tile([128, G + GS], F32, tag="lt")
            nc.vector.tensor_copy(out=lt, in_=ltp[:, 0:G + GS])
            gl0, gl1, el = lt[:, 0:1], lt[:, 1:2], lt[:, 2:6]
            gdiff = rtr.tile([128, 1], F32, tag="gdiff")
            nc.vector.tensor_sub(out=gdiff, in0=gl1, in1=gl0)
            gidx = rtr.tile([128, 1], F32, tag="gidx")
            nc.vector.tensor_scalar(out=gidx, in0=gdiff, scalar1=0.0,
                                    scalar2=None, op0=AL.is_gt)
            gabs = rtr.tile([128, 1], F32, tag="gabs")
            nc.scalar.activation(out=gabs, in_=gdiff, func=AF.Abs)
            gw = rtr.tile([128, 1], F32, tag="gw")
            nc.scalar.activation(out=gw, in_=gabs, func=AF.Sigmoid)
            m4 = rtr.tile([128, 1], F32, tag="m4")
            nc.vector.reduce_max(m4, el, axis=mybir.AxisListType.X)
            negm = rtr.tile([128, 1], F32, tag="negm")
            nc.vector.tensor_scalar_mul(out=negm, in0=m4, scalar1=-1.0)
            eexp = rtr.tile([128, GS], F32, tag="eexp")
            nc.scalar.activation(out=eexp, in_=el, func=AF.Exp,
                                 bias=negm, scale=1.0)
            ssum = rtr.tile([128, 1], F32, tag="ssum")
            nc.vector.reduce_sum(ssum, eexp, axis=mybir.AxisListType.X)
            srec = rtr.tile([128, 1], F32, tag="srec")
            nc.vector.reciprocal(out=srec, in_=ssum)
            ep = rtr.tile([128, GS], F32, tag="ep")
            nc.vector.tensor_scalar_mul(out=ep, in0=eexp, scalar1=srec)
            el8 = rtr.tile([128, 2 * GS], F32, tag="el8")
            nc.vector.tensor_copy(out=el8[:, 0:GS], in_=el)
            nc.vector.tensor_copy(out=el8[:, GS:2 * GS], in_=el)
            rank = rtr.tile([128, GS], F32, tag="rank")
            c2 = rtr.tile([128, GS], F32, tag="c2")
            nc.vector.tensor_tensor(out=rank, in0=el8[:, 1:5], in1=el, op=AL.is_gt)
            nc.vector.tensor_tensor(out=c2, in0=el8[:, 2:6], in1=el, op=AL.is_gt)
            nc.vector.tensor_add(out=rank, in0=rank, in1=c2)
            nc.vector.tensor_tensor(out=c2, in0=el8[:, 3:7], in1=el, op=AL.is_gt)
            nc.vector.tensor_add(out=rank, in0=rank, in1=c2)
            topm = rtr.tile([128, GS], F32, tag="topm")
            nc.vector.tensor_scalar(out=topm, in0=rank, scalar1=1.5,
                                    scalar2=None, op0=AL.is_le)
            wm = rtr.tile([128, GS], F32, tag="wm")
            nc.vector.tensor_mul(out=wm, in0=ep, in1=topm)
            wsum = rtr.tile([128, 1], F32, tag="wsum")
            nc.vector.reduce_sum(wsum, wm, axis=mybir.AxisListType.X)
            nc.vector.tensor_scalar_add(out=wsum, in0=wsum, scalar1=1e-7)
            wrec = rtr.tile([128, 1], F32, tag="wrec")
            nc.vector.reciprocal(out=wrec, in_=wsum)
            fwn = rtr.tile([128, GS], F32, tag="fwn")
            nc.vector.tensor_scalar_mul(out=fwn, in0=wm, scalar1=wrec)
            nc.vector.tensor_scalar_mul(out=fwn, in0=fwn, scalar1=gw)
            g0 = rtr.tile([128, 1], F32, tag="g0")
            nc.vector.tensor_scalar(out=g0, in0=gidx, scalar1=-1.0, scalar2=1.0,
                                    op0=AL.mult, op1=AL.add)
            dm = dm_all[:, tb, :]
            nc.vector.tensor_scalar_mul(out=dm[:, 0:GS], in0=fwn, scalar1=g0)
            nc.vector.tensor_scalar_mul(out=dm[:, GS:E], in0=fwn, scalar1=gidx)
            nc.vector.reduce_sum(sdm_all[:, tb:tb + 1], _f32(dm),
                                 axis=mybir.AxisListType.X)
            nc.tensor.matmul(load_ps[:, 0:E], ones_col, dm,
                             start=(tb == 0), stop=(tb == TB - 1))
            lsq = rtr.tile([128, G + GS], R32, tag="lsq")
            nc.scalar.square(out=lsq, in_=lt)
            nc.tensor.matmul(lsq_ps[:, 0:G + GS], ones_col, lsq,
                             start=(tb == 0), stop=(tb == TB - 1))
            nc.sync.dma_start(out=sdm_d[tb:tb + 1, :], in_=sdm_all[:, tb:tb + 1])

        rp_sb = tmp.tile([1, 16], F32, tag="rp")
        nc.vector.memset(rp_sb, 0.0)
        nc.vector.tensor_copy(out=rp_sb[:, 0:E], in_=load_ps[:, 0:E])
        nc.vector.tensor_copy(out=rp_sb[:, E:E + G + GS], in_=lsq_ps[:, 0:G + GS])
        nc.sync.dma_start(out=io["rpart"], in_=rp_sb)

        aexp_sb = singles.tile([128, E * A], R32)
        nc.sync.dma_start(out=aexp_sb, in_=io["aexpP"])
        # ---------- expert adapters -> hc^T (feature-major) ----------
        hcT_sb = big.tile([128, NL], R32)
        for tb in range(TB):
            he0 = ps_acc.tile([128, NL], F32, tag="acc", name="he0")
            he1 = ps_acc.tile([128, NL], F32, tag="acc", name="he1")
            nc.tensor.matmul(he0, pre_sb[:, tb * 128:(tb + 1) * 128],
                             aexp_sb[:, 0:512], start=True, stop=True)
            nc.tensor.matmul(he1, pre_sb[:, tb * 128:(tb + 1) * 128],
                             aexp_sb[:, 512:1024], start=True, stop=True)
            # per-expert LN over c (free dim), vectorized across experts
            hesq = tmp.tile([128, E * A], F32, tag="hesq", bufs=2)
            nc.scalar.square(out=hesq[:, 0:512], in_=he0)
            nc.scalar.square(out=hesq[:, 512:1024], in_=he1)
            mvs = tmp.tile([128, 4 * E], F32, tag="mvs", bufs=2)
            sums = mvs[:, 0:E]
            sqs = mvs[:, E:2 * E]
            rsd = mvs[:, 2 * E:3 * E]
            varv = mvs[:, 3 * E:4 * E]
            nc.vector.reduce_sum(sums[:, 0:4],
                                 he0.rearrange("p (e c) -> p e c", c=A),
                                 axis=mybir.AxisListType.X)
            nc.vector.reduce_sum(sums[:, 4:8],
                                 he1.rearrange("p (e c) -> p e c", c=A),
                                 axis=mybir.AxisListType.X)
            nc.vector.reduce_sum(sqs,
                                 hesq.rearrange("p (e c) -> p e c", c=A),
                                 axis=mybir.AxisListType.X)
            nc.vector.tensor_scalar_mul(out=sums, in0=sums, scalar1=1.0 / A)
            nc.vector.tensor_scalar_mul(out=sqs, in0=sqs, scalar1=1.0 / A)
            musq = tmp.tile([128, E], F32, tag="musq", bufs=2)
            nc.vector.tensor_mul(out=musq, in0=sums, in1=sums)
            nc.vector.tensor_sub(out=varv, in0=sqs, in1=musq)
            sdv = tmp.tile([128, E], F32, tag="sdv", bufs=2)
            nc.scalar.activation(out=sdv, in_=varv, func=AF.Sqrt,
                                 bias=epsP, scale=1.0)
            nc.vector.reciprocal(out=rsd, in_=sdv)
            he_ln = tmp.tile([128, E * A], F32, tag="heln", bufs=2)
            for e in range(E):
                srcp = he0[:, e * A:(e + 1) * A] if e < 4 else \
                    he1[:, (e - 4) * A:(e - 3) * A]
                nc.vector.tensor_scalar(out=he_ln[:, e * A:(e + 1) * A],
                                        in0=srcp,
                                        scalar1=sums[:, e:e + 1],
                                        scalar2=rsd[:, e:e + 1],
                                        op0=AL.subtract, op1=AL.mult)
            # ln_g_e / ln_b_e on the idle gpsimd engine (SBUF-only)
            nc.gpsimd.tensor_mul(out=he_ln, in0=he_ln, in1=lnge_bc)
            nc.gpsimd.tensor_add(out=he_ln, in0=he_ln, in1=lnbe_bc)
            hc_tok = tmp.tile([128, A], F32, tag="hctok", bufs=2)
            nc.vector.tensor_scalar_mul(out=hc_tok, in0=he_ln[:, 0:A],
                                        scalar1=_f32(dm_all)[:, tb, 0:1])
            acc = tmp.tile([128, A], F32, tag="hcacc", bufs=2)
            for e in range(1, E):
                seg = slice(e * A, (e + 1) * A)
                nc.vector.tensor_scalar_mul(out=acc, in0=he_ln[:, seg],
                                            scalar1=_f32(dm_all)[:, tb, e:e + 1])
                nc.vector.tensor_add(out=hc_tok, in0=hc_tok, in1=acc)
            tp = ps_tp.tile([128, 128], F32, tag="tp")
            nc.tensor.transpose(tp, hc_tok, ident)
            nc.vector.tensor_copy(out=hcT_sb[:, tb * 128:(tb + 1) * 128], in_=tp)

        # ---------- up/gate -> hidden (feature-major [H, NL]) ----------
        hidden_sb = big.tile([128, HC, NL], R32)
        wupT_r = io["wupT"].rearrange("(c p) h -> p c h", p=128)
        wgateT_r = io["wgateT"].rearrange("(c p) h -> p c h", p=128)
        for hc in range(HC):
            wu = xph.tile([128, DC, 128], R32, tag="wu", bufs=3)
            nc.sync.dma_start(out=wu, in_=wupT_r[:, :, hc * 128:(hc + 1) * 128])
            wg = xph.tile([128, DC, 128], R32, tag="wg", bufs=3)
            nc.sync.dma_start(out=wg, in_=wgateT_r[:, :, hc * 128:(hc + 1) * 128])
            up_ps = mm_tile()
            gt_ps = mm_tile()
            for dc in range(DC):
                nc.tensor.matmul(up_ps, wu[:, dc, :], xT_sb[:, dc, :],
                                 start=(dc == 0), stop=(dc == DC - 1))
            for dc in range(DC):
                nc.tensor.matmul(gt_ps, wg[:, dc, :], xT_sb[:, dc, :],
                                 start=(dc == 0), stop=(dc == DC - 1))
            sg = xph.tile([128, NL], F32, tag="sg", bufs=2)
            nc.scalar.activation(out=sg, in_=gt_ps, func=AF.Silu)
            nc.vector.tensor_mul(out=hidden_sb[:, hc, :], in0=sg, in1=up_ps)

        xph_cm.__exit__(None, None, None)

        # ai^T = LN(pre^T)  (feature-major; scores rhs)
        aiT_sb = big.tile([128, NL], R32)
        part_ln(pre_sb, aiT_sb)

        # ai token-major -> cin rows 0:512
        for tb in range(TB):
            tp = ps_tp.tile([128, 128], F32, tag="tp")
            nc.tensor.transpose(tp, _f32(aiT_sb)[:, tb * 128:(tb + 1) * 128],
                                ident)
            ai_tok = tmp.tile([128, 128], R32, tag="aitok")
            nc.vector.tensor_copy(out=ai_tok, in_=tp)
            nc.sync.dma_start(out=cin[tb * 128:(tb + 1) * 128, :], in_=ai_tok)

        # ---------- combo weights (D-sharded per group rank, x0.1) ----------
        c1_ps = mm_tile(256)
        c2_ps = mm_tile(256)
        for hc in range(HC):
            wep = wpool.tile([128, A], mybir.dt.bfloat16, tag="wep")
            nc.sync.dma_start(
                out=wep, in_=io["weproj"][hc * 128:(hc + 1) * 128, :])
            wap = wpool.tile([128, A], mybir.dt.bfloat16, tag="wap")
            nc.sync.dma_start(
                out=wap, in_=io["waproj"][hc * 128:(hc + 1) * 128, :])
            wcr = wpool.tile([128, 512], mybir.dt.bfloat16, tag="wcr")
            nc.sync.dma_start(
                out=wcr, in_=io["wcrhs"][hc * 128:(hc + 1) * 128, :])
            nc.tensor.matmul(c1_ps, wep, wcr[:, 0:256],
                             start=(hc == 0), stop=(hc == HC - 1))
            nc.tensor.matmul(c2_ps, wap, wcr[:, 256:512],
                             start=(hc == 0), stop=(hc == HC - 1))
        for cps, coff, tag in ((c1_ps, OFF_C1, "c1w"), (c2_ps, OFF_C2, "c2w")):
            c_sb = tmp.tile([128, 256], R32, tag=tag, bufs=1, name="c_sb")
            nc.vector.tensor_scalar_mul(out=c_sb, in0=cps, scalar1=0.1)
            nc.sync.dma_start(
                out=cin[coff:coff + 256, :].rearrange("(p x) c -> p (x c)", p=128),
                in_=c_sb)

        # ---------- ao^T = LN(hidden @ W_post.T)^T -> cin ----------
        wpost_sb = wpool.tile([128, HC, A], R32, tag="wpost", bufs=1)
        nc.sync.dma_start(out=wpost_sb,
                          in_=io["wpostT"].rearrange("(c p) a -> p c a", p=128))
        t1_ps = ps_acc.tile([128, NL], F32, tag="acc")
        for hc in range(HC):
            nc.tensor.matmul(t1_ps, wpost_sb[:, hc, :], hidden_sb[:, hc, :],
                             start=(hc == 0), stop=(hc == HC - 1))
        t1_sb = tmp.tile([128, NL], R32, tag="t1sb", bufs=1)
        nc.vector.tensor_copy(out=t1_sb, in_=t1_ps)
        aoT_sb = big.tile([128, NL], R32)
        part_ln(t1_sb, aoT_sb)
        nc.sync.dma_start(
            out=cin[OFF_AO:OFF_AO + ROWS_AO, :].rearrange(
                "(p x) c -> p (x c)", p=128),
            in_=aoT_sb)

        # ---------- the one collective ----------
        if single:
            # timing-sim stand-in: replicate own contribution into all slots
            for r in range(GRP):
                nc.sync.dma_start(
                    out=cout[r * CIN_ROWS:(r + 1) * CIN_ROWS, :], in_=cin[:])
        else:
            nc.gpsimd.collective_compute(
                "AllGather", mybir.AluOpType.bypass,
                replica_groups=[[0, 1, 2, 3], [4, 5, 6, 7]],
                ins=[cin[:].opt()], outs=[cout[:].opt()],
            )

        late = ctx.enter_context(tc.tile_pool(name="late", bufs=1))

        # ---------- gather results ----------
        aoT_all = []
        ai_tok_all = []
        c1_sb = []
        c2_sb = []
        for r in range(GRP):
            base = r * CIN_ROWS
            t_ao = late.tile([128, NL], R32, name=f"aoTall{r}")
            nc.sync.dma_start(
                out=t_ao,
                in_=cout[base + OFF_AO:base + OFF_AO + ROWS_AO, :].rearrange(
                    "(p x) c -> p (x c)", p=128))
            aoT_all.append(t_ao)
            t_ai = late.tile([128, TB, 128], R32, name=f"aitok{r}")
            nc.sync.dma_start(
                out=t_ai,
                in_=cout[base:base + ROWS_AI, :].rearrange(
                    "(t p) c -> p t c", p=128))
            ai_tok_all.append(t_ai)
            t_c1 = late.tile([128, 256], R32, name=f"c1rd{r}")
            nc.sync.dma_start(
                out=t_c1,
                in_=cout[base + OFF_C1:base + OFF_C1 + ROWS_C1, :].rearrange(
                    "(p x) c -> p (x c)", p=128))
            c1_sb.append(t_c1)
            t_c2 = late.tile([128, 256], R32, name=f"c2rd{r}")
            nc.sync.dma_start(
                out=t_c2,
                in_=cout[base + OFF_C2:base + OFF_C2 + ROWS_C2, :].rearrange(
                    "(p x) c -> p (x c)", p=128))
            c2_sb.append(t_c2)

        # ---------- adapter attention ----------
        adapt_ps = ps_acc.tile([128, NL], F32, tag="acc")
        nkb = GRP * TB
        for kb in range(nkb):
            r, j = divmod(kb, TB)
            sc_ps = mm_tile()
            nc.tensor.matmul(sc_ps, aoT_all[r][:, j * 128:(j + 1) * 128],
                             aiT_sb, start=True, stop=True)
            aw = late.tile([128, NL], R32, tag="aw", bufs=3)
            nc.vector.tensor_scalar(out=aw, in0=sc_ps, scalar1=5.0, scalar2=-5.0,
                                    op0=AL.min, op1=AL.max)
            nc.scalar.activation(out=aw, in_=_f32(aw), func=AF.Silu)
            nc.tensor.matmul(adapt_ps, ai_tok_all[r][:, j, :], aw,
                             start=(kb == 0), stop=(kb == nkb - 1))
        adapt_sb = late.tile([128, NL], R32)
        nc.vector.tensor_copy(out=adapt_sb, in_=adapt_ps)

        # ---------- sdm broadcast row (plain f32 matmul) ----------
        sdm_row = late.tile([1, NL], F32, tag="sdmrow")
        nc.sync.dma_start(out=sdm_row,
                          in_=sdm_d[:].rearrange("(o a) b -> o (a b)", o=1))
        sdm_bc_ps = ps_row.tile([128, NL], F32, tag="row2",
                                padded_shape=[128, NL])
        nc.tensor.matmul(sdm_bc_ps, ones_row, sdm_row, start=True, stop=True)
        sdm_bc = late.tile([128, NL], F32)
        nc.vector.tensor_copy(out=sdm_bc, in_=sdm_bc_ps)

        # ---------- down-proj + epilogue (fused) ----------
        outT_r = io["outT"].rearrange("(c p) n -> c p n", p=128)
        wdownT_r = io["wdownT"].rearrange("(c p) d -> p c d", p=128)
        for dc in range(DC):
            r, half = divmod(dc, 2)
            c1l = c1_sb[r][:, half * 128:(half + 1) * 128]
            c2l = c2_sb[r][:, half * 128:(half + 1) * 128]
            wd = wpool.tile([128, HC, 128], R32, tag="wd")
            nc.sync.dma_start(out=wd,
                              in_=wdownT_r[:, :, dc * 128:(dc + 1) * 128])
            sh_ps = ps_acc.tile([128, NL], F32, tag="acc", name="sh_ps")
            for hc in range(HC):
                nc.tensor.matmul(sh_ps, wd[:, hc, :], hidden_sb[:, hc, :],
                                 start=(hc == 0), stop=False)
            # + 0.1 * adapt @ combo2^T accumulated into the same bank
            nc.tensor.matmul(sh_ps, c2l, adapt_sb, start=False, stop=True)
            moe_ps = mm_tile()
            nc.tensor.matmul(moe_ps, c1l, hcT_sb, start=True, stop=True)
            ot = late.tile([128, NL], F32, tag="ot", bufs=2)
            nc.vector.tensor_mul(out=ot, in0=sh_ps, in1=sdm_bc)
            nc.vector.tensor_add(out=ot, in0=ot, in1=moe_ps)
            nc.sync.dma_start(out=outT_r[dc], in_=ot)


_CACHE = {}


def _get_program():
    if "nc" not in _CACHE:
        _CACHE["nc"] = build_program()
    return _CACHE["nc"]


def kernel(x, W_up, W_gate, W_down, W_pre, W_post, ln_g, ln_b, W_aproj,
           A_exp, ln_g_e, ln_b_e, W_eproj, W_oproj, W_rg, W_re):
    nc = _get_program()
    f = np.float32
    xf = np.ascontiguousarray(np.asarray(x, f).reshape(N, D))
    wupT = np.ascontiguousarray(np.asarray(W_up, f).T)      # [D,H]
    wgateT = np.ascontiguousarray(np.asarray(W_gate, f).T)  # [D,H]
    wdownT = np.ascontiguousarray(np.asarray(W_down, f).T)  # [H,D]
    wpreT = np.ascontiguousarray(np.asarray(W_pre, f).T)    # [D,A]
    wpostT = np.ascontiguousarray(np.asarray(W_post, f).T)  # [H,A]
    waproj = np.ascontiguousarray(
        np.asarray(W_aproj, f).astype(ml_dtypes.bfloat16))  # [H,A] bf16
    weproj = np.ascontiguousarray(
        np.asarray(W_eproj, f).astype(ml_dtypes.bfloat16))  # [H,A] bf16
    woprojT = np.ascontiguousarray(np.asarray(W_oproj, f).T)  # [H,D]
    # aexpP[a, e*A+c] = A_exp[e, c, a]
    aexpP = np.ascontiguousarray(
        np.asarray(A_exp, f).transpose(2, 0, 1).reshape(A, E * A))
    wrge = np.ascontiguousarray(
        np.concatenate([np.asarray(W_rg, f), np.asarray(W_re, f)], axis=0).T)
    lngb = np.ascontiguousarray(
        np.stack([np.asarray(ln_g, f), np.asarray(ln_b, f)], axis=1))  # [A,2]
    lngeP = np.ascontiguousarray(np.asarray(ln_g_e, f).reshape(1, E * A))
    lnbeP = np.ascontiguousarray(np.asarray(ln_b_e, f).reshape(1, E * A))

    in_maps = []
    wcrhs_cache = {}
    for c in range(NC):
        g = c % GRP
        if g not in wcrhs_cache:
            wcrhs_cache[g] = np.ascontiguousarray(np.concatenate(
                [woprojT[:, g * 256:(g + 1) * 256],
                 wdownT[:, g * 256:(g + 1) * 256]],
                axis=1).astype(ml_dtypes.bfloat16))
        xT_c = np.ascontiguousarray(xf[c * NL:(c + 1) * NL].T)  # [D, NL]
        in_maps.append({
            "xT": xT_c, "wupT": wupT, "wgateT": wgateT, "wdownT": wdownT,
            "wpreT": wpreT, "wpostT": wpostT, "waproj": waproj,
            "weproj": weproj, "wcrhs": wcrhs_cache[g], "aexpP": aexpP,
            "wrge": wrge, "lngb": lngb, "lngeP": lngeP, "lnbeP": lnbeP,
        })

    res = bass_utils.run_bass_kernel_spmd(nc, in_maps, core_ids=list(range(NC)))

    out = np.empty((N, D), f)
    load = np.zeros(E, np.float64)
    sq = np.zeros(6, np.float64)
    for c in range(NC):
        out[c * NL:(c + 1) * NL] = res.results[c]["outT"].T
        rp = res.results[c]["rpart"][0]
        load += rp[0:E]
        sq += rp[E:E + 6]
    target = load.sum() / E
    router_loss = 0.001 * (np.mean((load - target) ** 2)
                           + sq[0:G].sum() / (N * G)
                           + sq[G:G + GS].sum() / (N * GS))
    return out.reshape(B, S, D), np.float32(router_loss)
